# revision 5
# baseline (speedup 1.0000x reference)
"""Trainium2 Bass kernel for nn_GAT_Solution (GNN message passing, 8-core data parallel).

Sharding: batch dim across 8 cores (4 batches each); small params replicated.
Host does integer index prep + weight layout/scale folding only; all float
compute runs on device.

v2 design (vs baseline):
  - K projection runs in fp8e4 (eg gathered embeddings and Wk both f8 with
    power-of-2 scales folded into host-prepared weights); same PE rate but
    halves the eg HBM traffic.
  - The mix-MLP hidden layer is ONE DoubleRow fp8 matmul per 512-col slice:
    tile0 = combo x prod8 (prod8 = q*k scaled by PS, written f8 by the DVE),
    tile1 = rank-1 edge-cost term (ec8 row rides in partition 0 of a
    once-zeroed carrier; stationary tile1 is w1bo in row 0). This removes the
    80 separate rank-1 ec matmuls entirely; relu evacuation applies the
    1/(PS*CS) dequant via the activation scale.
  - The softmax weighted sum runs in NATURAL node layout: w_n [128,8,S] f16
    comes straight out of the softmax (no transpose-back, no partition
    broadcasts, no PE ones-matmuls); pm_s = eg_nat8_s * w_n[:,:,s] broadcast
    along the inner free axis; f16 add tree; 8 f16 PE transposes + one DVE
    copy produce accT for the GRU. The embedding fp8 scale ES cancels via
    WihT pre-divided by ES on host.
  - GRU stays f16 (fp8 gates measurably break the 2e-2 budget).
  - Elementwise ops spread across DVE/Scalar/GpSimd by measured budget.
"""

import os
import numpy as np
import ml_dtypes

S, B, G, E, NH, KD, MSH = 10, 32, 1000, 128, 8, 16, 16
NCORES = 8
BC = B // NCORES          # 4 batches per core
GP = 1024                 # padded node count

# fp8 power-of-2 scales (folded into host-prepared weights)
ES = 32.0                 # embedding scale (egT8, eg_nat8)
SWK = 64.0                # Wk scale
PS = 16.0                 # prod8 scale (folded into WqT)
CS = 16.0                 # combo scale
SE = 64.0                 # edge-cost scale

_RUN_STATE = {}


# --------------------------------------------------------------------------
# device program
# --------------------------------------------------------------------------

def _build_program():
    import contextlib
    import concourse.bass as bass
    import concourse.bacc as bacc
    import concourse.tile as tile
    from concourse import mybir

    dt = mybir.dt
    AF = mybir.ActivationFunctionType
    OP = mybir.AluOpType
    AX = mybir.AxisListType
    DR = mybir.MatmulPerfMode.DoubleRow

    nc = bacc.Bacc("TRN2", target_bir_lowering=False, debug=False,
                   enable_asserts=False)

    def inp(name, shape, dtype):
        return nc.dram_tensor(name, list(shape), dtype, kind="ExternalInput").ap()

    embT   = inp("embT",   (BC, 128, G), dt.float16)
    egT8   = inp("egT8",   (BC, S, 128, GP), dt.float8e4)
    egn16  = inp("egn16",  (BC, S, 128, 8, 128), dt.float16)
    ecr8   = inp("ecr8",   (BC, S, GP), dt.float8e4)
    succn  = inp("succn",  (BC, 128, 8, S), dt.float32)
    cinv   = inp("cinv",   (BC, 128, 8, S), dt.float32)
    soldT  = inp("soldT",  (BC, 128, G), dt.float16)
    invc   = inp("invc",   (BC, 128, S), dt.float32)
    c0invc = inp("c0invc", (BC, 128, S), dt.float32)
    WqT    = inp("WqT",    (128, 128), dt.float16)
    WkT8   = inp("WkT8",   (128, 128), dt.float8e4)
    cw8    = inp("cw8",    (128, 2, 128), dt.float8e4)
    b1f    = inp("b1f",    (128, 1), dt.float32)
    coef   = inp("coef",   (128, 32), dt.float16)
    ident  = inp("ident",  (128, 128), dt.float32)
    ident16 = inp("ident16", (128, 128), dt.float16)
    WihT   = inp("WihT",   (128, 384), dt.float16)
    WhhT   = inp("WhhT",   (128, 384), dt.float16)
    gbsum  = inp("gbsum",  (128, 2), dt.float32)
    bihn   = inp("bihn",   (128, 1), dt.float32)
    bhhn   = inp("bhhn",   (128, 1), dt.float32)

    outT = nc.dram_tensor("outT", [2, BC, 128, G], dt.float16,
                          kind="ExternalOutput").ap()

    DQ = 1.0 / (PS * CS)   # mix dequant, applied in the relu activation

    with tile.TileContext(nc) as tc:
        with contextlib.ExitStack() as ctx:
            cpool = ctx.enter_context(tc.tile_pool(name="consts", bufs=1))
            io = ctx.enter_context(tc.tile_pool(name="io", bufs=2))
            gat = ctx.enter_context(tc.tile_pool(name="gat", bufs=3))
            egn = ctx.enter_context(tc.tile_pool(name="egn", bufs=22))
            work = ctx.enter_context(tc.tile_pool(name="work", bufs=2))
            ms1p = ctx.enter_context(tc.tile_pool(name="ms1p", bufs=3))
            sm = ctx.enter_context(tc.tile_pool(name="sm", bufs=2))
            rowa = ctx.enter_context(tc.tile_pool(name="rowa", bufs=2))
            pmp = ctx.enter_context(tc.tile_pool(name="pmp", bufs=11))
            tre = ctx.enter_context(tc.tile_pool(name="tre", bufs=1))
            gru = ctx.enter_context(tc.tile_pool(name="gru", bufs=1))
            psa = ctx.enter_context(
                tc.tile_pool(name="psa", bufs=1, space="PSUM"))
            psb = ctx.enter_context(
                tc.tile_pool(name="psb", bufs=2, space="PSUM"))
            psc = ctx.enter_context(
                tc.tile_pool(name="psc", bufs=1, space="PSUM"))

            def const(ap_, dtype, tag):
                t = cpool.tile(list(ap_.shape), dtype, tag=tag)
                nc.sync.dma_start(t[:], ap_)
                return t

            WqT_t = const(WqT, dt.float16, "cWqT")
            WkT8_t = const(WkT8, dt.float8e4, "cWkT8")
            cw8_t = const(cw8, dt.float8e4, "ccw8")
            b1f_t = const(b1f, dt.float32, "cb1f")
            coef_t = const(coef, dt.float16, "ccoef")
            ident_t = const(ident, dt.float32, "cident")
            ident16_t = const(ident16, dt.float16, "cident16")
            WihT_t = const(WihT, dt.float16, "cWih")
            WhhT_t = const(WhhT, dt.float16, "cWhh")
            gbsum_t = const(gbsum, dt.float32, "cgb")
            bihn_t = const(bihn, dt.float32, "cbihn")
            bhhn_t = const(bhhn, dt.float32, "cbhhn")

            # DoubleRow rhs ring: 3 persistent buffers (manually cycled so
            # the carrier zeros survive reuse).  Only [:,0,:] (prod8) and
            # [0:1,1,:] (ec row) are rewritten per use; the tile framework's
            # dependency tracking on the shared tensors enforces the ring.
            dr_bufs = [cpool.tile([128, 2, GP], dt.float8e4, tag=f"drb{i}",
                                  name=f"drb{i}")
                       for i in range(3)]
            for t in dr_bufs:
                nc.vector.memset(t[:, 1, :], 0.0)
            dr_ctr = [0]

            def mix_head(b):
                """Batch-b prologue: Q projection + the tail's small DMAs."""
                embT_t = io.tile([128, G], dt.float16, tag="embT")
                nc.sync.dma_start(embT_t[:], embT[b])

                qt_ps = psb.tile([128, GP], dt.float32, tag="mm")
                nc.tensor.matmul(qt_ps[:, 0:512], WqT_t[:], embT_t[:, 0:512],
                                 start=True, stop=True)
                nc.tensor.matmul(qt_ps[:, 512:G], WqT_t[:],
                                 embT_t[:, 512:G], start=True, stop=True)
                qt16 = work.tile([128, GP], dt.float16, tag="qt16")
                nc.vector.memset(qt16[:, G:GP], 0.0)
                nc.scalar.copy(qt16[:, 0:G], qt_ps[:, 0:G])

                st = {}
                st["invb"] = sm.tile([128, S], dt.float32, tag="invb",
                                     name=f"invb_{b}")
                nc.sync.dma_start(st["invb"][:], invc[b])
                st["c0b"] = sm.tile([128, S], dt.float32, tag="c0b",
                                    name=f"c0b_{b}")
                nc.sync.dma_start(st["c0b"][:], c0invc[b])
                st["sn"] = sm.tile([128, 8, S], dt.float32, tag="succn",
                                   name=f"sn_{b}")
                nc.sync.dma_start(st["sn"][:], succn[b])
                st["cnv"] = sm.tile([128, 8, S], dt.float32, tag="cinv",
                                    name=f"cnv_{b}")
                nc.sync.dma_start(st["cnv"][:], cinv[b])
                st["sold"] = gru.tile([128, GP], dt.float16,
                                      tag=f"sold{b % 2}", name=f"sold_{b}")
                nc.sync.dma_start(st["sold"][:, 0:G], soldT[b])
                nc.vector.memset(st["sold"][:, G:GP], 0.0)

                # eq depends only on the successor indices -- compute it here,
                # off the tail's critical chain
                eq = sm.tile([128, 8, S, S], dt.float32, tag="eq",
                             name=f"eq_{b}")
                nc.vector.tensor_tensor(
                    eq[:],
                    st["sn"][:].unsqueeze(3).broadcast_to([128, 8, S, S]),
                    st["sn"][:].unsqueeze(2).broadcast_to([128, 8, S, S]),
                    OP.is_equal)
                st["eq"] = eq

                cost_sb = rowa.tile([S, GP], dt.float32, tag="costsb")
                nc.vector.memset(cost_sb[:, G:GP], 0.0)
                st["cost_sb"] = cost_sb
                st["qt16"] = qt16
                st["egn"] = []
                return st

            def mix_group(b, st, grp):
                """Mix MLP for a group of <=3 solutions.  Per solution: f8 K
                matmuls, prod8 (DVE) into the DoubleRow rhs tile, one DR
                matmul per 512-slice (combo + rank-1 ec fused), relu evac
                with dequant scale, then the coef e-row matmuls."""
                qt16 = st["qt16"]
                cost_sb = st["cost_sb"]
                e_ps = psa.tile([96, GP], dt.float32, tag="e1")
                for gs, s in enumerate(grp):
                    eg_t = gat.tile([128, GP], dt.float8e4, tag="eg")
                    nc.sync.dma_start(eg_t[:], egT8[b, s])
                    egn_t = egn.tile([128, 8, 128], dt.float16, tag="egn")
                    nc.sync.dma_start(egn_t[:], egn16[b, s])
                    st["egn"].append(egn_t)

                    dr_t = dr_bufs[dr_ctr[0] % 3]
                    dr_ctr[0] += 1
                    nc.sync.dma_start(dr_t[0:1, 1, :], ecr8[b, s].unsqueeze(0))

                    kg_ps = psb.tile([128, GP], dt.float32, tag="mm")
                    nc.tensor.matmul(kg_ps[:, 0:512], WkT8_t[:],
                                     eg_t[:, 0:512], start=True, stop=True)
                    nc.tensor.matmul(kg_ps[:, 512:GP], WkT8_t[:],
                                     eg_t[:, 512:GP], start=True, stop=True)
                    nc.vector.tensor_mul(dr_t[:, 0, :], qt16[:], kg_ps[:])

                    ms1_ps = psb.tile([128, GP], dt.float32, tag="mm")
                    for hh in range(2):
                        sl = slice(hh * 512, (hh + 1) * 512)
                        nc.tensor.matmul(ms1_ps[:, sl], cw8_t[:],
                                         dr_t[:, :, sl], start=True,
                                         stop=True, perf_mode=DR)
                    ms1 = ms1p.tile([128, GP], dt.float16, tag="ms1")
                    nc.scalar.activation(ms1[:], ms1_ps[:], AF.Relu,
                                         bias=b1f_t[:], scale=DQ)
                    base = gs * 32
                    for hh in range(2):
                        sl = slice(hh * 512, (hh + 1) * 512)
                        nc.tensor.matmul(e_ps[base:base + 32, sl],
                                         coef_t[:], ms1[:, sl],
                                         start=True, stop=True)
                np_ = (len(grp) - 1) * 32 + 1
                e_sb = work.tile([65, GP], dt.float32, tag="erow")
                nc.scalar.copy(e_sb[0:np_, 0:G], e_ps[0:np_, 0:G])
                for gs, s in enumerate(grp):
                    nc.sync.dma_start(cost_sb[s:s + 1, 0:G],
                                      e_sb[gs * 32:gs * 32 + 1, 0:G])

            def tail_cn(b, st):
                """Raw e rows -> natural layout [128, 8, S] (PE transposes,
                emitted first so they aren't queued behind the next mix)."""
                cost_sb = st["cost_sb"]
                cn_ps = psc.tile([128, 8 * S], dt.float32, tag="ps2",
                                 name=f"cn_{b}")
                for blk in range(8):
                    nc.tensor.transpose(
                        cn_ps[:, blk * S:(blk + 1) * S],
                        cost_sb[:, blk * 128:(blk + 1) * 128],
                        ident_t[0:S, 0:S])
                st["cn_ps"] = cn_ps

            def tail_softmax(b, st):
                """Merged softmax over the <=10 edges per node,
                cost = e/costs + C0/costs.  Emits w_n [128, 8, S] f16 in
                natural node layout (partition = t%128, blk = t//128)."""
                invb, c0b, cnv = st["invb"], st["c0b"], st["cnv"]
                cn_ps = st["cn_ps"]
                craw = sm.tile([128, 8, S], dt.float32, tag="craw")
                nc.vector.tensor_mul(
                    craw[:], cn_ps[:].rearrange("p (a b) -> p a b", a=8),
                    invb[:].unsqueeze(1).broadcast_to([128, 8, S]))
                cost_n = sm.tile([128, 8, S], dt.float32, tag="costn")
                nc.vector.tensor_add(
                    cost_n[:], craw[:],
                    c0b[:].unsqueeze(1).broadcast_to([128, 8, S]))

                # ---- softmax with duplicate-successor merge (no max shift:
                # |merged cost| < 1 by construction)
                eq = st["eq"]
                mm_ = sm.tile([128, 8, S, S], dt.float32, tag="mmul")
                nc.gpsimd.tensor_mul(
                    mm_[:], eq[:],
                    cost_n[:].unsqueeze(2).broadcast_to([128, 8, S, S]))
                m_t = sm.tile([128, 8, S], dt.float32, tag="mt")
                nc.vector.tensor_reduce(m_t[:], mm_[:], AX.X, OP.add)
                p_t = sm.tile([128, 8, S], dt.float32, tag="pt")
                nc.scalar.activation(p_t[:], m_t[:], AF.Exp)
                pc = sm.tile([128, 8, S], dt.float32, tag="pc")
                nc.vector.tensor_mul(pc[:], p_t[:], cnv[:])
                z_t = sm.tile([128, 8], dt.float32, tag="zt")
                nc.vector.tensor_reduce(z_t[:], pc[:], AX.X, OP.add)
                zr = sm.tile([128, 8], dt.float32, tag="zr")
                nc.vector.reciprocal_approx_fast(zr[:], z_t[:])
                w_n = sm.tile([128, 8, S], dt.float16, tag="wn")
                nc.vector.tensor_mul(
                    w_n[:], pc[:],
                    zr[:].unsqueeze(2).broadcast_to([128, 8, S]))
                st["w_n"] = w_n

            def tail_wsum(b, st):
                """Weighted sum of successor embeddings, natural layout:
                pm_s = eg_nat8_s * w_n[:,:,s] (broadcast along d), f16 add
                tree, then 8 f16 transposes + one DVE copy -> accT [128, GP]
                (values carry the ES embedding scale; WihT is pre-divided)."""
                w_n = st["w_n"]
                egn_tiles = st["egn"]

                pm = [None] * S
                for s in range(6, S):      # GpSimd muls first: they start
                    pm_t = pmp.tile([128, 8, 128], dt.float16, tag="pm",
                                    name=f"pmg_{s}")
                    nc.gpsimd.tensor_mul(
                        pm_t[:], egn_tiles[s][:],
                        w_n[:, :, s].unsqueeze(2).broadcast_to([128, 8, 128]))
                    pm[s] = pm_t
                for s in range(6):
                    pm_t = pmp.tile([128, 8, 128], dt.float16, tag="pm",
                                    name=f"pmv_{s}")
                    nc.vector.tensor_mul(
                        pm_t[:], egn_tiles[s][:],
                        w_n[:, :, s].unsqueeze(2).broadcast_to([128, 8, 128]))
                    pm[s] = pm_t
                t67 = tre.tile([128, 8, 128], dt.float16, tag="t67")
                nc.gpsimd.tensor_add(t67[:], pm[6][:], pm[7][:])
                t89 = tre.tile([128, 8, 128], dt.float16, tag="t89")
                nc.gpsimd.tensor_add(t89[:], pm[8][:], pm[9][:])
                t01 = tre.tile([128, 8, 128], dt.float16, tag="t01")
                nc.vector.tensor_add(t01[:], pm[0][:], pm[1][:])
                t23 = tre.tile([128, 8, 128], dt.float16, tag="t23")
                nc.vector.tensor_add(t23[:], pm[2][:], pm[3][:])
                t45 = tre.tile([128, 8, 128], dt.float16, tag="t45")
                nc.vector.tensor_add(t45[:], pm[4][:], pm[5][:])
                a0 = tre.tile([128, 8, 128], dt.float16, tag="a0")
                nc.vector.tensor_add(a0[:], t01[:], t23[:])
                a1 = tre.tile([128, 8, 128], dt.float16, tag="a1")
                nc.gpsimd.tensor_add(a1[:], t67[:], t89[:])
                a2 = tre.tile([128, 8, 128], dt.float16, tag="t23")
                nc.vector.tensor_add(a2[:], a0[:], t45[:])
                accn = tre.tile([128, 8, 128], dt.float16, tag="accn")
                nc.vector.tensor_add(accn[:], a2[:], a1[:])

                tr_ps = psc.tile([128, GP], dt.float16, tag="ps2",
                                 name=f"tr_{b}")
                for blk in range(8):
                    nc.tensor.transpose(
                        tr_ps[:, blk * 128:(blk + 1) * 128],
                        accn[:, blk, :], ident16_t[:])
                acc = gru.tile([128, GP], dt.float16, tag="accT")
                nc.vector.tensor_copy(acc[:], tr_ps[:])
                st["acc"] = acc

            def tail_gru(b, st):
                """GRU cell + elu epilogue (all f16 matmuls; acc carries the
                ES scale, cancelled by the host-scaled WihT)."""
                acc = st["acc"]
                sold_t = st["sold"]

                # r/z gates use different PSUM pools so they run in parallel
                # sigmoid(x) = 0.5*(1 + tanh(x/2)): the gates are computed
                # as t = tanh(0.5*x + 0.5*b) (all acts stay in the
                # exp_and_others table set -> zero table reloads), and the
                # 0.5/+1 affines fold into fused scalar_tensor_tensor ops.
                gates = []
                for gidx, pool in ((0, psc), (1, psb)):   # r, z fused ih+hh
                    g_ps = pool.tile([128, GP], dt.float32,
                                     tag="ps2" if pool is psc else "mm",
                                     name=f"g{gidx}_{b}")
                    wsl = slice(gidx * 128, (gidx + 1) * 128)
                    for sl in (slice(0, 512), slice(512, GP)):
                        nc.tensor.matmul(g_ps[:, sl], WihT_t[:, wsl],
                                         acc[:, sl], start=True, stop=False)
                        nc.tensor.matmul(g_ps[:, sl], WhhT_t[:, wsl],
                                         sold_t[:, sl], start=False, stop=True)
                    gt = gru.tile([128, GP], dt.float16, tag=f"gate{gidx}")
                    nc.scalar.activation(gt[:], g_ps[:], AF.Tanh,
                                         bias=gbsum_t[:, gidx:gidx + 1],
                                         scale=0.5)
                    gates.append(gt)
                r_t, z_g = gates

                ghn_ps = psc.tile([128, GP], dt.float32, tag="ps2")
                for sl in (slice(0, 512), slice(512, GP)):
                    nc.tensor.matmul(ghn_ps[:, sl], WhhT_t[:, 256:384],
                                     sold_t[:, sl], start=True, stop=True)
                ghs = gru.tile([128, GP], dt.float16, tag="ghs")
                nc.vector.tensor_scalar(ghs[:], ghn_ps[:], bhhn_t[:], 0.5,
                                        OP.add, OP.mult)
                gin_ps = psb.tile([128, GP], dt.float32, tag="mm")
                for sl in (slice(0, 512), slice(512, GP)):
                    nc.tensor.matmul(gin_ps[:, sl], WihT_t[:, 256:384],
                                     acc[:, sl], start=True, stop=True)
                rh = gru.tile([128, GP], dt.float16, tag="rh")
                nc.vector.scalar_tensor_tensor(rh[:], r_t[:], 1.0, ghs[:],
                                               OP.add, OP.mult)
                tn = gru.tile([128, GP], dt.float32, tag="tn")
                nc.vector.tensor_add(tn[:], gin_ps[:], rh[:])
                n_t = gru.tile([128, GP], dt.float16, tag="nt")
                nc.scalar.activation(n_t[:], tn[:], AF.Tanh, bias=bihn_t[:])

                # new = n + sigmoid_z*(h-n) = n + 0.5*(t_z+1)*(h-n)
                d_t = gru.tile([128, GP], dt.float16, tag="dt")
                nc.vector.tensor_sub(d_t[:], sold_t[:], n_t[:])
                zd = gru.tile([128, GP], dt.float16, tag="zd")
                nc.vector.scalar_tensor_tensor(zd[:], z_g[:], 1.0, d_t[:],
                                               OP.add, OP.mult)
                new_t = gru.tile([128, GP], dt.float16, tag="newt")
                nc.vector.scalar_tensor_tensor(new_t[:], zd[:], 0.5, n_t[:],
                                               OP.mult, OP.add)
                nc.sync.dma_start(outT[1, b], new_t[:, 0:G])

                # elu(new) = relu(new) + exp(min(new,0)) - 1
                m0 = gru.tile([128, GP], dt.float16, tag="dt2")
                nc.gpsimd.tensor_scalar_min(m0[:], new_t[:], 0.0)
                ex = gru.tile([128, GP], dt.float16, tag="rh2")
                nc.scalar.activation(ex[:], m0[:], AF.Exp)
                rl = gru.tile([128, GP], dt.float16, tag="ghs2")
                nc.vector.tensor_sub(rl[:], new_t[:], m0[:])
                ex1 = gru.tile([128, GP], dt.float16, tag="nt2")
                nc.gpsimd.tensor_scalar_add(ex1[:], ex[:], -1.0)
                el = gru.tile([128, GP], dt.float16, tag="el")
                nc.vector.tensor_add(el[:], ex1[:], rl[:])
                nc.sync.dma_start(outT[0, b], el[:, 0:G])

            # software pipeline: interleave batch b+1's PE-heavy mix chunks
            # with batch b's DVE/Scalar-heavy tail chunks so every engine's
            # in-order queue alternates between the two batches.
            GRPS = [(0, 1, 2), (3, 4), (5, 6, 7), (8, 9)]
            states = {0: mix_head(0)}
            for grp in GRPS:
                mix_group(0, states[0], grp)
            for b in range(BC):
                nb = b + 1
                tail_cn(b, states[b])
                if nb < BC:
                    states[nb] = mix_head(nb)
                    mix_group(nb, states[nb], GRPS[0])
                tail_softmax(b, states[b])
                if nb < BC:
                    mix_group(nb, states[nb], GRPS[1])
                tail_wsum(b, states[b])
                if nb < BC:
                    mix_group(nb, states[nb], GRPS[2])
                tail_gru(b, states.pop(b))
                if nb < BC:
                    mix_group(nb, states[nb], GRPS[3])

    nc.compile()
    return nc


# --------------------------------------------------------------------------
# host prep (integer index work + layout/scale staging only)
# --------------------------------------------------------------------------

def _host_prep(node_embed, solutions, costs, dist, solution_embed_old,
               Wq, Wk, mix1_weight, mix1_bias, mix2_weight, mix2_bias,
               norm_head_w, gru_w_ih, gru_w_hh, gru_b_ih, gru_b_hh):
    f32 = np.float32
    f16 = np.float16
    f8 = ml_dtypes.float8_e4m3

    sol = np.asarray(solutions).astype(np.int64)
    nxt = np.roll(sol, -1, axis=-1)
    # succ[s,b,i]: successor of node i in tour (s,b)
    succ = np.zeros((S, B, G), dtype=np.int64)
    s_idx = np.arange(S)[:, None, None]
    b_idx = np.arange(B)[None, :, None]
    succ[s_idx, b_idx, sol] = nxt

    node_embed = np.asarray(node_embed, f32)
    dist = np.asarray(dist, f32)
    sold = np.asarray(solution_embed_old, f32)
    costs = np.asarray(costs, f32)

    Wq = np.asarray(Wq, f32); Wk = np.asarray(Wk, f32)
    m1w = np.asarray(mix1_weight, f32)   # [H, 2, M]
    m1b = np.asarray(mix1_bias, f32)     # [H, M]
    m2w = np.asarray(mix2_weight, f32)   # [H, M, 1]
    m2b = np.asarray(mix2_bias, f32)     # [H, 1]
    nhw = np.asarray(norm_head_w, f32)   # [H]
    wih = np.asarray(gru_w_ih, f32); whh = np.asarray(gru_w_hh, f32)
    bih = np.asarray(gru_b_ih, f32); bhh = np.asarray(gru_b_hh, f32)

    hm_h = np.repeat(np.arange(NH), MSH)          # head of each (h,m) slot
    dp_h = np.repeat(np.arange(NH), KD)           # head of each d' slot
    combo = np.where(dp_h[:, None] == hm_h[None, :],
                     (m1w[:, 0, :].reshape(-1) / 16.0)[None, :], 0.0)
    w1bo = m1w[:, 1, :].reshape(128)
    # DoubleRow stationary: tile0 = combo*CS, tile1 = w1bo carrier (row 0)
    cw8 = np.zeros((128, 2, 128), f8)
    cw8[:, 0, :] = (combo * CS).astype(f8)
    cw8[0, 1, :] = (w1bo * (CS * PS / SE)).astype(f8)

    coef = np.zeros((128, 32), f32)
    coef[:, 0] = (m2w[:, :, 0] * nhw[:, None]).reshape(128)
    c0 = float(np.dot(m2b[:, 0], nhw))
    gb = bih + bhh

    consts = dict(
        WqT=(np.ascontiguousarray(Wq.T) * (PS / (ES * SWK))).astype(f16),
        WkT8=(np.ascontiguousarray(Wk.T) * SWK).astype(f8),
        cw8=cw8,
        b1f=m1b.reshape(128, 1).astype(f32),
        coef=coef.astype(f16),
        ident=np.eye(128, dtype=f32),
        ident16=np.eye(128, dtype=f16),
        WihT=np.ascontiguousarray(wih.T).astype(f16),   # [128, 384]
        WhhT=np.ascontiguousarray(whh.T).astype(f16),
        gbsum=np.stack([gb[0:128] * 0.5, gb[128:256] * 0.5], axis=1).astype(f32),
        bihn=bih[256:384].reshape(128, 1).astype(f32),
        bhhn=bhh[256:384].reshape(128, 1).astype(f32),
    )

    iv = np.arange(G)
    in_maps = []
    for c in range(NCORES):
        bs = slice(c * BC, (c + 1) * BC)
        ne = node_embed[bs]                        # [BC, G, E] f32
        ne8 = (ne * ES).astype(f8)                 # fp8 with ES scale
        sc = succ[:, bs, :]                        # [S, BC, G]

        egT8_ = np.zeros((BC, S, 128, GP), f8)
        egn16_ = np.zeros((BC, S, 128, 8, 128), f16)
        ecr8_ = np.zeros((BC, S, GP), f8)
        succn = np.zeros((BC, 128, 8, S), f32)
        cinv_ = np.ones((BC, 128, 8, S), f32)
        for bb in range(BC):
            scb = sc[:, bb, :]                     # [S, G]
            eqh = scb[None, :, :] == scb[:, None, :]
            cnt = eqh.sum(1)                       # [S, G] dup counts
            for s in range(S):
                sv = scb[s]                        # [G]
                g8 = ne8[bb][sv]                   # [G, 128] f8
                egT8_[bb, s, :, 0:G] = g8.T
                # natural layout: [t%128, t//128, d], zero-padded to 1024
                gpad = np.zeros((GP, 128), f16)
                gpad[0:G] = ne[bb][sv].astype(f16)
                egn16_[bb, s] = gpad.reshape(8, 128, 128).transpose(1, 0, 2)
                ecr8_[bb, s, 0:G] = (
                    dist[bs][bb][iv, sv] * SE).astype(f8)
                succn[bb, :, :, s] = 2000.0 + s
                succn[bb, iv % 128, iv // 128, s] = sv
                cinv_[bb, iv % 128, iv // 128, s] = 1.0 / cnt[s]

        im = dict(consts)
        im.update(
            embT=np.ascontiguousarray(
                ne.transpose(0, 2, 1)).astype(f16),
            egT8=egT8_,
            egn16=egn16_,
            ecr8=ecr8_,
            succn=succn,
            cinv=cinv_,
            soldT=np.ascontiguousarray(
                sold[bs].transpose(0, 2, 1)).astype(f16),
            invc=np.ascontiguousarray(np.broadcast_to(
                (1.0 / costs[:, bs]).T[:, None, :], (BC, 128, S))).astype(f32),
            c0invc=np.ascontiguousarray(np.broadcast_to(
                (c0 / costs[:, bs]).T[:, None, :], (BC, 128, S))).astype(f32),
        )
        in_maps.append(im)
    return in_maps


# --------------------------------------------------------------------------
# runner (mirrors concourse.bass2jax.run_bass_via_pjrt, but caches the jitted
# executable and keeps inputs device-resident so repeated runs can be timed)
# --------------------------------------------------------------------------

def _get_runner():
    if "runner" in _RUN_STATE:
        return _RUN_STATE["runner"]

    import jax
    from jax.sharding import Mesh, PartitionSpec
    from jax.experimental.shard_map import shard_map
    from concourse import mybir
    from concourse.bass2jax import (_bass_exec_p, install_neuronx_cc_hook,
                                    partition_id_tensor)

    if "nc" not in _RUN_STATE:
        _RUN_STATE["nc"] = _build_program()
    nc = _RUN_STATE["nc"]
    install_neuronx_cc_hook()

    pid_name = (nc.partition_id_tensor.name
                if nc.partition_id_tensor is not None else None)
    in_names, out_names, out_avals = [], [], []
    for alloc in nc.m.functions[0].allocations:
        if not isinstance(alloc, mybir.MemoryLocationSet):
            continue
        name = alloc.memorylocations[0].name
        if alloc.kind == "ExternalInput":
            if name != pid_name:
                in_names.append(name)
        elif alloc.kind == "ExternalOutput":
            out_names.append(name)
            out_avals.append(jax.core.ShapedArray(
                tuple(alloc.tensor_shape), mybir.dt.np(alloc.dtype)))
    n_params = len(in_names)
    all_names = in_names + out_names
    if pid_name is not None:
        all_names = all_names + [pid_name]

    def _body(*args):
        operands = list(args)
        if pid_name is not None:
            operands.append(partition_id_tensor())
        outs = _bass_exec_p.bind(
            *operands,
            out_avals=tuple(out_avals),
            in_names=tuple(all_names),
            out_names=tuple(out_names),
            lowering_input_output_aliases=(),
            sim_require_finite=True,
            sim_require_nnan=True,
            nc=nc,
        )
        return tuple(outs)

    devices = jax.devices()[:NCORES]
    mesh = Mesh(np.asarray(devices), ("core",))
    n_outs = len(out_names)
    sharded = jax.jit(
        shard_map(_body, mesh=mesh,
                  in_specs=(PartitionSpec("core"),) * (n_params + n_outs),
                  out_specs=(PartitionSpec("core"),) * n_outs,
                  check_rep=False),
        keep_unused=True,
    )

    runner = dict(fn=sharded, in_names=in_names, out_names=out_names,
                  out_avals=out_avals, mesh=mesh)
    _RUN_STATE["runner"] = runner
    return runner


def _device_args(runner, in_maps):
    import jax
    from jax.sharding import NamedSharding, PartitionSpec
    sh = NamedSharding(runner["mesh"], PartitionSpec("core"))
    args = []
    for i, name in enumerate(runner["in_names"]):
        arr = np.concatenate([np.asarray(m[name]) for m in in_maps], axis=0)
        args.append(jax.device_put(arr, sh))
    for av in runner["out_avals"]:
        z = np.zeros((NCORES * av.shape[0], *av.shape[1:]), av.dtype)
        args.append(jax.device_put(z, sh))
    return args


def _run(in_maps):
    runner = _get_runner()
    args = _device_args(runner, in_maps)
    outs = runner["fn"](*args)
    return {name: np.asarray(outs[i])
            for i, name in enumerate(runner["out_names"])}


def bench(in_maps, iters=10):
    """Time repeated executions with device-resident inputs; returns
    (min_s, mean_s) per execution (includes axon RPC overhead)."""
    import time as _time
    import jax
    runner = _get_runner()
    args = _device_args(runner, in_maps)
    outs = runner["fn"](*args)           # warm-up/compile
    jax.block_until_ready(outs)
    times = []
    for _ in range(iters):
        t0 = _time.perf_counter()
        outs = runner["fn"](*args)
        jax.block_until_ready(outs)
        times.append(_time.perf_counter() - t0)
    return min(times), sum(times) / len(times)


# --------------------------------------------------------------------------
# entry point
# --------------------------------------------------------------------------

def kernel(**inputs):
    in_maps = _host_prep(**inputs)
    res = _run(in_maps)
    full = res["outT"].reshape(NCORES, 2, BC, 128, G).astype(np.float32)
    full = np.concatenate([full[c] for c in range(NCORES)], axis=1)
    full = np.ascontiguousarray(full.transpose(0, 1, 3, 2))  # [2, B, G, E]
    return (full[0], full[1])


# revision 6
# speedup vs baseline: 1.4330x; 1.4330x over previous
"""Trainium2 Bass kernel for nn_GAT_Solution (GNN message passing, 8-core data parallel).

Sharding: batch dim across 8 cores (4 batches each); small params replicated.
Host does integer index prep + weight layout/scale folding only; all float
compute runs on device.

v2 design (vs baseline):
  - K projection runs in fp8e4 (eg gathered embeddings and Wk both f8 with
    power-of-2 scales folded into host-prepared weights); same PE rate but
    halves the eg HBM traffic.
  - The mix-MLP hidden layer is ONE DoubleRow fp8 matmul per 512-col slice:
    tile0 = combo x prod8 (prod8 = q*k scaled by PS, written f8 by the DVE),
    tile1 = rank-1 edge-cost term (ec8 row rides in partition 0 of a
    once-zeroed carrier; stationary tile1 is w1bo in row 0). This removes the
    80 separate rank-1 ec matmuls entirely; relu evacuation applies the
    1/(PS*CS) dequant via the activation scale.
  - The softmax weighted sum runs in NATURAL node layout: w_n [128,8,S] f16
    comes straight out of the softmax (no transpose-back, no partition
    broadcasts, no PE ones-matmuls); pm_s = eg_nat8_s * w_n[:,:,s] broadcast
    along the inner free axis; f16 add tree; 8 f16 PE transposes + one DVE
    copy produce accT for the GRU. The embedding fp8 scale ES cancels via
    WihT pre-divided by ES on host.
  - GRU stays f16 (fp8 gates measurably break the 2e-2 budget).
  - Elementwise ops spread across DVE/Scalar/GpSimd by measured budget.
"""

import os
import numpy as np
import ml_dtypes

S, B, G, E, NH, KD, MSH = 10, 32, 1000, 128, 8, 16, 16
NCORES = 8
BC = B // NCORES          # 4 batches per core
GP = 1024                 # padded node count

# fp8 power-of-2 scales (folded into host-prepared weights)
ES = 32.0                 # embedding scale (egT8, eg_nat8)
SWK = 64.0                # Wk scale
PS = 16.0                 # prod8 scale (folded into WqT)
CS = 16.0                 # combo scale
SE = 64.0                 # edge-cost scale

_RUN_STATE = {}


# --------------------------------------------------------------------------
# device program
# --------------------------------------------------------------------------

def _build_program():
    import contextlib
    import concourse.bass as bass
    import concourse.bacc as bacc
    import concourse.tile as tile
    from concourse import mybir

    dt = mybir.dt
    AF = mybir.ActivationFunctionType
    OP = mybir.AluOpType
    AX = mybir.AxisListType
    DR = mybir.MatmulPerfMode.DoubleRow

    nc = bacc.Bacc("TRN2", target_bir_lowering=False, debug=False,
                   enable_asserts=False)

    def inp(name, shape, dtype):
        return nc.dram_tensor(name, list(shape), dtype, kind="ExternalInput").ap()

    embT   = inp("embT",   (BC, 128, G), dt.float16)
    egT8   = inp("egT8",   (BC, S, 128, GP), dt.float8e4)
    egn16  = inp("egn16",  (BC, S, 128, 8, 128), dt.float16)
    ecr8   = inp("ecr8",   (BC, S, GP), dt.float8e4)
    succn  = inp("succn",  (BC, 128, 8, S), dt.float32)
    cinv   = inp("cinv",   (BC, 128, 8, S), dt.float32)
    soldT  = inp("soldT",  (BC, 128, G), dt.float16)
    invc   = inp("invc",   (BC, 128, S), dt.float32)
    c0invc = inp("c0invc", (BC, 128, S), dt.float32)
    WqT    = inp("WqT",    (128, 128), dt.float16)
    WkT8   = inp("WkT8",   (128, 128), dt.float8e4)
    cw8    = inp("cw8",    (128, 2, 128), dt.float8e4)
    b1f    = inp("b1f",    (128, 1), dt.float32)
    coef   = inp("coef",   (128, 32), dt.float16)
    ident  = inp("ident",  (128, 128), dt.float32)
    ident16 = inp("ident16", (128, 128), dt.float16)
    WihT   = inp("WihT",   (128, 384), dt.float16)
    WhhT   = inp("WhhT",   (128, 384), dt.float16)
    gbsum  = inp("gbsum",  (128, 2), dt.float32)
    bihn   = inp("bihn",   (128, 1), dt.float32)
    bhhn   = inp("bhhn",   (128, 1), dt.float32)

    outT = nc.dram_tensor("outT", [2, BC, 128, G], dt.float16,
                          kind="ExternalOutput").ap()

    DQ = 1.0 / (PS * CS)   # mix dequant, applied in the relu activation

    with tile.TileContext(nc) as tc:
        with contextlib.ExitStack() as ctx:
            cpool = ctx.enter_context(tc.tile_pool(name="consts", bufs=1))
            io = ctx.enter_context(tc.tile_pool(name="io", bufs=2))
            gat = ctx.enter_context(tc.tile_pool(name="gat", bufs=3))
            egn = ctx.enter_context(tc.tile_pool(name="egn", bufs=22))
            work = ctx.enter_context(tc.tile_pool(name="work", bufs=2))
            ms1p = ctx.enter_context(tc.tile_pool(name="ms1p", bufs=3))
            sm = ctx.enter_context(tc.tile_pool(name="sm", bufs=2))
            rowa = ctx.enter_context(tc.tile_pool(name="rowa", bufs=2))
            pmp = ctx.enter_context(tc.tile_pool(name="pmp", bufs=11))
            tre = ctx.enter_context(tc.tile_pool(name="tre", bufs=1))
            gru = ctx.enter_context(tc.tile_pool(name="gru", bufs=1))
            psa = ctx.enter_context(
                tc.tile_pool(name="psa", bufs=1, space="PSUM"))
            psb = ctx.enter_context(
                tc.tile_pool(name="psb", bufs=2, space="PSUM"))
            psc = ctx.enter_context(
                tc.tile_pool(name="psc", bufs=1, space="PSUM"))

            def const(ap_, dtype, tag):
                t = cpool.tile(list(ap_.shape), dtype, tag=tag)
                nc.sync.dma_start(t[:], ap_)
                return t

            WqT_t = const(WqT, dt.float16, "cWqT")
            WkT8_t = const(WkT8, dt.float8e4, "cWkT8")
            cw8_t = const(cw8, dt.float8e4, "ccw8")
            b1f_t = const(b1f, dt.float32, "cb1f")
            coef_t = const(coef, dt.float16, "ccoef")
            ident_t = const(ident, dt.float32, "cident")
            ident16_t = const(ident16, dt.float16, "cident16")
            WihT_t = const(WihT, dt.float16, "cWih")
            WhhT_t = const(WhhT, dt.float16, "cWhh")
            gbsum_t = const(gbsum, dt.float32, "cgb")
            bihn_t = const(bihn, dt.float32, "cbihn")
            bhhn_t = const(bhhn, dt.float32, "cbhhn")

            # DoubleRow rhs ring: 3 persistent buffers (manually cycled so
            # the carrier zeros survive reuse).  Only [:,0,:] (prod8) and
            # [0:1,1,:] (ec row) are rewritten per use; the tile framework's
            # dependency tracking on the shared tensors enforces the ring.
            dr_bufs = [cpool.tile([128, 2, GP], dt.float8e4, tag=f"drb{i}",
                                  name=f"drb{i}")
                       for i in range(3)]
            for t in dr_bufs:
                nc.vector.memset(t[:, 1, :], 0.0)
            dr_ctr = [0]

            def mix_head(b):
                """Batch-b prologue: Q projection + the tail's small DMAs."""
                embT_t = io.tile([128, G], dt.float16, tag="embT")
                nc.sync.dma_start(embT_t[:], embT[b])

                qt_ps = psb.tile([128, GP], dt.float32, tag="mm")
                nc.tensor.matmul(qt_ps[:, 0:512], WqT_t[:], embT_t[:, 0:512],
                                 start=True, stop=True)
                nc.tensor.matmul(qt_ps[:, 512:G], WqT_t[:],
                                 embT_t[:, 512:G], start=True, stop=True)
                qt16 = work.tile([128, GP], dt.float16, tag="qt16")
                nc.vector.memset(qt16[:, G:GP], 0.0)
                nc.scalar.copy(qt16[:, 0:G], qt_ps[:, 0:G])

                st = {}
                st["invb"] = sm.tile([128, S], dt.float32, tag="invb",
                                     name=f"invb_{b}")
                nc.sync.dma_start(st["invb"][:], invc[b])
                st["c0b"] = sm.tile([128, S], dt.float32, tag="c0b",
                                    name=f"c0b_{b}")
                nc.sync.dma_start(st["c0b"][:], c0invc[b])
                st["sn"] = sm.tile([128, 8, S], dt.float32, tag="succn",
                                   name=f"sn_{b}")
                nc.sync.dma_start(st["sn"][:], succn[b])
                st["cnv"] = sm.tile([128, 8, S], dt.float32, tag="cinv",
                                    name=f"cnv_{b}")
                nc.sync.dma_start(st["cnv"][:], cinv[b])
                st["sold"] = gru.tile([128, GP], dt.float16,
                                      tag=f"sold{b % 2}", name=f"sold_{b}")
                nc.sync.dma_start(st["sold"][:, 0:G], soldT[b])
                nc.vector.memset(st["sold"][:, G:GP], 0.0)

                # eq depends only on the successor indices -- compute it here,
                # off the tail's critical chain
                eq = sm.tile([128, 8, S, S], dt.float32, tag="eq",
                             name=f"eq_{b}")
                nc.vector.tensor_tensor(
                    eq[:],
                    st["sn"][:].unsqueeze(3).broadcast_to([128, 8, S, S]),
                    st["sn"][:].unsqueeze(2).broadcast_to([128, 8, S, S]),
                    OP.is_equal)
                st["eq"] = eq

                cost_sb = rowa.tile([S, GP], dt.float32, tag="costsb")
                nc.vector.memset(cost_sb[:, G:GP], 0.0)
                st["cost_sb"] = cost_sb
                st["qt16"] = qt16
                st["egn"] = []
                return st

            def mix_group(b, st, grp):
                """Mix MLP for a group of <=3 solutions.  Per solution: f8 K
                matmuls, prod8 (DVE) into the DoubleRow rhs tile, one DR
                matmul per 512-slice (combo + rank-1 ec fused), relu evac
                with dequant scale, then the coef e-row matmuls."""
                qt16 = st["qt16"]
                cost_sb = st["cost_sb"]
                e_ps = psa.tile([96, GP], dt.float32, tag="e1")
                for gs, s in enumerate(grp):
                    eg_t = gat.tile([128, GP], dt.float8e4, tag="eg")
                    nc.sync.dma_start(eg_t[:], egT8[b, s])
                    egn_t = egn.tile([128, 8, 128], dt.float16, tag="egn")
                    nc.sync.dma_start(egn_t[:], egn16[b, s])
                    st["egn"].append(egn_t)

                    dr_t = dr_bufs[dr_ctr[0] % 3]
                    dr_ctr[0] += 1
                    nc.sync.dma_start(dr_t[0:1, 1, :], ecr8[b, s].unsqueeze(0))

                    kg_ps = psb.tile([128, GP], dt.float32, tag="mm")
                    nc.tensor.matmul(kg_ps[:, 0:512], WkT8_t[:],
                                     eg_t[:, 0:512], start=True, stop=True)
                    nc.tensor.matmul(kg_ps[:, 512:GP], WkT8_t[:],
                                     eg_t[:, 512:GP], start=True, stop=True)
                    nc.vector.tensor_mul(dr_t[:, 0, :], qt16[:], kg_ps[:])

                    ms1_ps = psb.tile([128, GP], dt.float32, tag="mm")
                    for hh in range(2):
                        sl = slice(hh * 512, (hh + 1) * 512)
                        nc.tensor.matmul(ms1_ps[:, sl], cw8_t[:],
                                         dr_t[:, :, sl], start=True,
                                         stop=True, perf_mode=DR)
                    ms1 = ms1p.tile([128, GP], dt.float16, tag="ms1")
                    nc.scalar.activation(ms1[:], ms1_ps[:], AF.Relu,
                                         bias=b1f_t[:], scale=DQ)
                    base = gs * 32
                    for hh in range(2):
                        sl = slice(hh * 512, (hh + 1) * 512)
                        nc.tensor.matmul(e_ps[base:base + 32, sl],
                                         coef_t[:], ms1[:, sl],
                                         start=True, stop=True)
                np_ = (len(grp) - 1) * 32 + 1
                e_sb = work.tile([65, GP], dt.float32, tag="erow")
                nc.scalar.copy(e_sb[0:np_, 0:G], e_ps[0:np_, 0:G])
                for gs, s in enumerate(grp):
                    nc.sync.dma_start(cost_sb[s:s + 1, 0:G],
                                      e_sb[gs * 32:gs * 32 + 1, 0:G])

            def tail_cn(b, st):
                """Raw e rows -> natural layout [128, 8, S] (PE transposes,
                emitted first so they aren't queued behind the next mix)."""
                cost_sb = st["cost_sb"]
                cn_ps = psc.tile([128, 8 * S], dt.float32, tag="ps2",
                                 name=f"cn_{b}")
                for blk in range(8):
                    nc.tensor.transpose(
                        cn_ps[:, blk * S:(blk + 1) * S],
                        cost_sb[:, blk * 128:(blk + 1) * 128],
                        ident_t[0:S, 0:S])
                st["cn_ps"] = cn_ps

            def tail_softmax(b, st):
                """Merged softmax over the <=10 edges per node,
                cost = e/costs + C0/costs.  Emits w_n [128, 8, S] f16 in
                natural node layout (partition = t%128, blk = t//128)."""
                invb, c0b, cnv = st["invb"], st["c0b"], st["cnv"]
                cn_ps = st["cn_ps"]
                craw = sm.tile([128, 8, S], dt.float32, tag="craw")
                nc.vector.tensor_mul(
                    craw[:], cn_ps[:].rearrange("p (a b) -> p a b", a=8),
                    invb[:].unsqueeze(1).broadcast_to([128, 8, S]))
                cost_n = sm.tile([128, 8, S], dt.float32, tag="costn")
                nc.vector.tensor_add(
                    cost_n[:], craw[:],
                    c0b[:].unsqueeze(1).broadcast_to([128, 8, S]))

                # ---- softmax with duplicate-successor merge (no max shift:
                # |merged cost| < 1 by construction)
                eq = st["eq"]
                mm_ = sm.tile([128, 8, S, S], dt.float32, tag="mmul")
                nc.gpsimd.tensor_mul(
                    mm_[:], eq[:],
                    cost_n[:].unsqueeze(2).broadcast_to([128, 8, S, S]))
                m_t = sm.tile([128, 8, S], dt.float32, tag="mt")
                nc.vector.tensor_reduce(m_t[:], mm_[:], AX.X, OP.add)
                p_t = sm.tile([128, 8, S], dt.float32, tag="pt")
                nc.scalar.activation(p_t[:], m_t[:], AF.Exp)
                pc = sm.tile([128, 8, S], dt.float32, tag="pc")
                nc.vector.tensor_mul(pc[:], p_t[:], cnv[:])
                z_t = sm.tile([128, 8], dt.float32, tag="zt")
                nc.vector.tensor_reduce(z_t[:], pc[:], AX.X, OP.add)
                zr = sm.tile([128, 8], dt.float32, tag="zr")
                nc.vector.reciprocal_approx_fast(zr[:], z_t[:])
                w_n = sm.tile([128, 8, S], dt.float16, tag="wn")
                nc.vector.tensor_mul(
                    w_n[:], pc[:],
                    zr[:].unsqueeze(2).broadcast_to([128, 8, S]))
                st["w_n"] = w_n

            def tail_wsum(b, st):
                """Weighted sum of successor embeddings, natural layout:
                pm_s = eg_nat8_s * w_n[:,:,s] (broadcast along d), f16 add
                tree, then 8 f16 transposes + one DVE copy -> accT [128, GP]
                (values carry the ES embedding scale; WihT is pre-divided)."""
                w_n = st["w_n"]
                egn_tiles = st["egn"]

                pm = [None] * S
                for s in range(6, S):      # GpSimd muls first: they start
                    pm_t = pmp.tile([128, 8, 128], dt.float16, tag="pm",
                                    name=f"pmg_{s}")
                    nc.gpsimd.tensor_mul(
                        pm_t[:], egn_tiles[s][:],
                        w_n[:, :, s].unsqueeze(2).broadcast_to([128, 8, 128]))
                    pm[s] = pm_t
                for s in range(6):
                    pm_t = pmp.tile([128, 8, 128], dt.float16, tag="pm",
                                    name=f"pmv_{s}")
                    nc.vector.tensor_mul(
                        pm_t[:], egn_tiles[s][:],
                        w_n[:, :, s].unsqueeze(2).broadcast_to([128, 8, 128]))
                    pm[s] = pm_t
                t67 = tre.tile([128, 8, 128], dt.float16, tag="t67")
                nc.gpsimd.tensor_add(t67[:], pm[6][:], pm[7][:])
                t89 = tre.tile([128, 8, 128], dt.float16, tag="t89")
                nc.vector.tensor_add(t89[:], pm[8][:], pm[9][:])
                t01 = tre.tile([128, 8, 128], dt.float16, tag="t01")
                nc.vector.tensor_add(t01[:], pm[0][:], pm[1][:])
                t23 = tre.tile([128, 8, 128], dt.float16, tag="t23")
                nc.vector.tensor_add(t23[:], pm[2][:], pm[3][:])
                t45 = tre.tile([128, 8, 128], dt.float16, tag="t45")
                nc.vector.tensor_add(t45[:], pm[4][:], pm[5][:])
                a0 = tre.tile([128, 8, 128], dt.float16, tag="a0")
                nc.vector.tensor_add(a0[:], t01[:], t23[:])
                a1 = tre.tile([128, 8, 128], dt.float16, tag="a1")
                nc.vector.tensor_add(a1[:], t67[:], t89[:])
                a2 = tre.tile([128, 8, 128], dt.float16, tag="t23")
                nc.vector.tensor_add(a2[:], a0[:], t45[:])
                accn = tre.tile([128, 8, 128], dt.float16, tag="accn")
                nc.vector.tensor_add(accn[:], a2[:], a1[:])

                tr_ps = psc.tile([128, GP], dt.float16, tag="ps2",
                                 name=f"tr_{b}")
                for blk in range(8):
                    nc.tensor.transpose(
                        tr_ps[:, blk * 128:(blk + 1) * 128],
                        accn[:, blk, :], ident16_t[:])
                acc = gru.tile([128, GP], dt.float16, tag="accT")
                nc.vector.tensor_copy(acc[:], tr_ps[:])
                st["acc"] = acc

            def tail_gru(b, st):
                """GRU cell + elu epilogue (all f16 matmuls; acc carries the
                ES scale, cancelled by the host-scaled WihT)."""
                acc = st["acc"]
                sold_t = st["sold"]

                # r/z gates use different PSUM pools so they run in parallel
                # sigmoid(x) = 0.5*(1 + tanh(x/2)): the gates are computed
                # as t = tanh(0.5*x + 0.5*b) (all acts stay in the
                # exp_and_others table set -> zero table reloads), and the
                # 0.5/+1 affines fold into fused scalar_tensor_tensor ops.
                gates = []
                for gidx, pool in ((0, psc), (1, psb)):   # r, z fused ih+hh
                    g_ps = pool.tile([128, GP], dt.float32,
                                     tag="ps2" if pool is psc else "mm",
                                     name=f"g{gidx}_{b}")
                    wsl = slice(gidx * 128, (gidx + 1) * 128)
                    for sl in (slice(0, 512), slice(512, GP)):
                        nc.tensor.matmul(g_ps[:, sl], WihT_t[:, wsl],
                                         acc[:, sl], start=True, stop=False)
                        nc.tensor.matmul(g_ps[:, sl], WhhT_t[:, wsl],
                                         sold_t[:, sl], start=False, stop=True)
                    gt = gru.tile([128, GP], dt.float16, tag=f"gate{gidx}")
                    nc.scalar.activation(gt[:], g_ps[:], AF.Tanh,
                                         bias=gbsum_t[:, gidx:gidx + 1],
                                         scale=0.5)
                    gates.append(gt)
                r_t, z_g = gates

                ghn_ps = psc.tile([128, GP], dt.float32, tag="ps2")
                for sl in (slice(0, 512), slice(512, GP)):
                    nc.tensor.matmul(ghn_ps[:, sl], WhhT_t[:, 256:384],
                                     sold_t[:, sl], start=True, stop=True)
                ghs = gru.tile([128, GP], dt.float16, tag="ghs")
                nc.vector.tensor_scalar(ghs[:], ghn_ps[:], bhhn_t[:], 0.5,
                                        OP.add, OP.mult)
                gin_ps = psb.tile([128, GP], dt.float32, tag="mm")
                for sl in (slice(0, 512), slice(512, GP)):
                    nc.tensor.matmul(gin_ps[:, sl], WihT_t[:, 256:384],
                                     acc[:, sl], start=True, stop=True)
                rh = gru.tile([128, GP], dt.float16, tag="rh")
                nc.vector.scalar_tensor_tensor(rh[:], r_t[:], 1.0, ghs[:],
                                               OP.add, OP.mult)
                tn = gru.tile([128, GP], dt.float32, tag="tn")
                nc.vector.tensor_add(tn[:], gin_ps[:], rh[:])
                n_t = gru.tile([128, GP], dt.float16, tag="nt")
                nc.scalar.activation(n_t[:], tn[:], AF.Tanh, bias=bihn_t[:])

                # new = n + sigmoid_z*(h-n) = n + 0.5*(t_z+1)*(h-n)
                d_t = gru.tile([128, GP], dt.float16, tag="dt")
                nc.vector.tensor_sub(d_t[:], sold_t[:], n_t[:])
                zd = gru.tile([128, GP], dt.float16, tag="zd")
                nc.vector.scalar_tensor_tensor(zd[:], z_g[:], 1.0, d_t[:],
                                               OP.add, OP.mult)
                new_t = gru.tile([128, GP], dt.float16, tag="newt")
                nc.vector.scalar_tensor_tensor(new_t[:], zd[:], 0.5, n_t[:],
                                               OP.mult, OP.add)
                nc.sync.dma_start(outT[1, b], new_t[:, 0:G])

                # elu(new) = relu(new) + exp(min(new,0)) - 1
                m0 = gru.tile([128, GP], dt.float16, tag="dt2")
                nc.vector.tensor_scalar_min(m0[:], new_t[:], 0.0)
                ex = gru.tile([128, GP], dt.float16, tag="rh2")
                nc.scalar.activation(ex[:], m0[:], AF.Exp)
                rl = gru.tile([128, GP], dt.float16, tag="ghs2")
                nc.vector.tensor_sub(rl[:], new_t[:], m0[:])
                ex1 = gru.tile([128, GP], dt.float16, tag="nt2")
                nc.vector.tensor_scalar_add(ex1[:], ex[:], -1.0)
                el = gru.tile([128, GP], dt.float16, tag="el")
                nc.vector.tensor_add(el[:], ex1[:], rl[:])
                nc.sync.dma_start(outT[0, b], el[:, 0:G])

            # software pipeline: interleave batch b+1's PE-heavy mix chunks
            # with batch b's DVE/Scalar-heavy tail chunks so every engine's
            # in-order queue alternates between the two batches.
            GRPS = [(0, 1, 2), (3, 4), (5, 6, 7), (8, 9)]
            states = {0: mix_head(0)}
            for grp in GRPS:
                mix_group(0, states[0], grp)
            for b in range(BC):
                nb = b + 1
                tail_cn(b, states[b])
                if nb < BC:
                    states[nb] = mix_head(nb)
                    mix_group(nb, states[nb], GRPS[0])
                tail_softmax(b, states[b])
                if nb < BC:
                    mix_group(nb, states[nb], GRPS[1])
                tail_wsum(b, states[b])
                if nb < BC:
                    mix_group(nb, states[nb], GRPS[2])
                tail_gru(b, states.pop(b))
                if nb < BC:
                    mix_group(nb, states[nb], GRPS[3])

    nc.compile()
    return nc


# --------------------------------------------------------------------------
# host prep (integer index work + layout/scale staging only)
# --------------------------------------------------------------------------

def _host_prep(node_embed, solutions, costs, dist, solution_embed_old,
               Wq, Wk, mix1_weight, mix1_bias, mix2_weight, mix2_bias,
               norm_head_w, gru_w_ih, gru_w_hh, gru_b_ih, gru_b_hh):
    f32 = np.float32
    f16 = np.float16
    f8 = ml_dtypes.float8_e4m3

    sol = np.asarray(solutions).astype(np.int64)
    nxt = np.roll(sol, -1, axis=-1)
    # succ[s,b,i]: successor of node i in tour (s,b)
    succ = np.zeros((S, B, G), dtype=np.int64)
    s_idx = np.arange(S)[:, None, None]
    b_idx = np.arange(B)[None, :, None]
    succ[s_idx, b_idx, sol] = nxt

    node_embed = np.asarray(node_embed, f32)
    dist = np.asarray(dist, f32)
    sold = np.asarray(solution_embed_old, f32)
    costs = np.asarray(costs, f32)

    Wq = np.asarray(Wq, f32); Wk = np.asarray(Wk, f32)
    m1w = np.asarray(mix1_weight, f32)   # [H, 2, M]
    m1b = np.asarray(mix1_bias, f32)     # [H, M]
    m2w = np.asarray(mix2_weight, f32)   # [H, M, 1]
    m2b = np.asarray(mix2_bias, f32)     # [H, 1]
    nhw = np.asarray(norm_head_w, f32)   # [H]
    wih = np.asarray(gru_w_ih, f32); whh = np.asarray(gru_w_hh, f32)
    bih = np.asarray(gru_b_ih, f32); bhh = np.asarray(gru_b_hh, f32)

    hm_h = np.repeat(np.arange(NH), MSH)          # head of each (h,m) slot
    dp_h = np.repeat(np.arange(NH), KD)           # head of each d' slot
    combo = np.where(dp_h[:, None] == hm_h[None, :],
                     (m1w[:, 0, :].reshape(-1) / 16.0)[None, :], 0.0)
    w1bo = m1w[:, 1, :].reshape(128)
    # DoubleRow stationary: tile0 = combo*CS, tile1 = w1bo carrier (row 0)
    cw8 = np.zeros((128, 2, 128), f8)
    cw8[:, 0, :] = (combo * CS).astype(f8)
    cw8[0, 1, :] = (w1bo * (CS * PS / SE)).astype(f8)

    coef = np.zeros((128, 32), f32)
    coef[:, 0] = (m2w[:, :, 0] * nhw[:, None]).reshape(128)
    c0 = float(np.dot(m2b[:, 0], nhw))
    gb = bih + bhh

    consts = dict(
        WqT=(np.ascontiguousarray(Wq.T) * (PS / (ES * SWK))).astype(f16),
        WkT8=(np.ascontiguousarray(Wk.T) * SWK).astype(f8),
        cw8=cw8,
        b1f=m1b.reshape(128, 1).astype(f32),
        coef=coef.astype(f16),
        ident=np.eye(128, dtype=f32),
        ident16=np.eye(128, dtype=f16),
        WihT=np.ascontiguousarray(wih.T).astype(f16),   # [128, 384]
        WhhT=np.ascontiguousarray(whh.T).astype(f16),
        gbsum=np.stack([gb[0:128] * 0.5, gb[128:256] * 0.5], axis=1).astype(f32),
        bihn=bih[256:384].reshape(128, 1).astype(f32),
        bhhn=bhh[256:384].reshape(128, 1).astype(f32),
    )

    iv = np.arange(G)
    in_maps = []
    for c in range(NCORES):
        bs = slice(c * BC, (c + 1) * BC)
        ne = node_embed[bs]                        # [BC, G, E] f32
        ne8 = (ne * ES).astype(f8)                 # fp8 with ES scale
        sc = succ[:, bs, :]                        # [S, BC, G]

        egT8_ = np.zeros((BC, S, 128, GP), f8)
        egn16_ = np.zeros((BC, S, 128, 8, 128), f16)
        ecr8_ = np.zeros((BC, S, GP), f8)
        succn = np.zeros((BC, 128, 8, S), f32)
        cinv_ = np.ones((BC, 128, 8, S), f32)
        for bb in range(BC):
            scb = sc[:, bb, :]                     # [S, G]
            eqh = scb[None, :, :] == scb[:, None, :]
            cnt = eqh.sum(1)                       # [S, G] dup counts
            for s in range(S):
                sv = scb[s]                        # [G]
                g8 = ne8[bb][sv]                   # [G, 128] f8
                egT8_[bb, s, :, 0:G] = g8.T
                # natural layout: [t%128, t//128, d], zero-padded to 1024
                gpad = np.zeros((GP, 128), f16)
                gpad[0:G] = ne[bb][sv].astype(f16)
                egn16_[bb, s] = gpad.reshape(8, 128, 128).transpose(1, 0, 2)
                ecr8_[bb, s, 0:G] = (
                    dist[bs][bb][iv, sv] * SE).astype(f8)
                succn[bb, :, :, s] = 2000.0 + s
                succn[bb, iv % 128, iv // 128, s] = sv
                cinv_[bb, iv % 128, iv // 128, s] = 1.0 / cnt[s]

        im = dict(consts)
        im.update(
            embT=np.ascontiguousarray(
                ne.transpose(0, 2, 1)).astype(f16),
            egT8=egT8_,
            egn16=egn16_,
            ecr8=ecr8_,
            succn=succn,
            cinv=cinv_,
            soldT=np.ascontiguousarray(
                sold[bs].transpose(0, 2, 1)).astype(f16),
            invc=np.ascontiguousarray(np.broadcast_to(
                (1.0 / costs[:, bs]).T[:, None, :], (BC, 128, S))).astype(f32),
            c0invc=np.ascontiguousarray(np.broadcast_to(
                (c0 / costs[:, bs]).T[:, None, :], (BC, 128, S))).astype(f32),
        )
        in_maps.append(im)
    return in_maps


# --------------------------------------------------------------------------
# runner (mirrors concourse.bass2jax.run_bass_via_pjrt, but caches the jitted
# executable and keeps inputs device-resident so repeated runs can be timed)
# --------------------------------------------------------------------------

def _get_runner():
    if "runner" in _RUN_STATE:
        return _RUN_STATE["runner"]

    import jax
    from jax.sharding import Mesh, PartitionSpec
    from jax.experimental.shard_map import shard_map
    from concourse import mybir
    from concourse.bass2jax import (_bass_exec_p, install_neuronx_cc_hook,
                                    partition_id_tensor)

    if "nc" not in _RUN_STATE:
        _RUN_STATE["nc"] = _build_program()
    nc = _RUN_STATE["nc"]
    install_neuronx_cc_hook()

    pid_name = (nc.partition_id_tensor.name
                if nc.partition_id_tensor is not None else None)
    in_names, out_names, out_avals = [], [], []
    for alloc in nc.m.functions[0].allocations:
        if not isinstance(alloc, mybir.MemoryLocationSet):
            continue
        name = alloc.memorylocations[0].name
        if alloc.kind == "ExternalInput":
            if name != pid_name:
                in_names.append(name)
        elif alloc.kind == "ExternalOutput":
            out_names.append(name)
            out_avals.append(jax.core.ShapedArray(
                tuple(alloc.tensor_shape), mybir.dt.np(alloc.dtype)))
    n_params = len(in_names)
    all_names = in_names + out_names
    if pid_name is not None:
        all_names = all_names + [pid_name]

    def _body(*args):
        operands = list(args)
        if pid_name is not None:
            operands.append(partition_id_tensor())
        outs = _bass_exec_p.bind(
            *operands,
            out_avals=tuple(out_avals),
            in_names=tuple(all_names),
            out_names=tuple(out_names),
            lowering_input_output_aliases=(),
            sim_require_finite=True,
            sim_require_nnan=True,
            nc=nc,
        )
        return tuple(outs)

    devices = jax.devices()[:NCORES]
    mesh = Mesh(np.asarray(devices), ("core",))
    n_outs = len(out_names)
    sharded = jax.jit(
        shard_map(_body, mesh=mesh,
                  in_specs=(PartitionSpec("core"),) * (n_params + n_outs),
                  out_specs=(PartitionSpec("core"),) * n_outs,
                  check_rep=False),
        keep_unused=True,
    )

    runner = dict(fn=sharded, in_names=in_names, out_names=out_names,
                  out_avals=out_avals, mesh=mesh)
    _RUN_STATE["runner"] = runner
    return runner


def _device_args(runner, in_maps):
    import jax
    from jax.sharding import NamedSharding, PartitionSpec
    sh = NamedSharding(runner["mesh"], PartitionSpec("core"))
    args = []
    for i, name in enumerate(runner["in_names"]):
        arr = np.concatenate([np.asarray(m[name]) for m in in_maps], axis=0)
        args.append(jax.device_put(arr, sh))
    for av in runner["out_avals"]:
        z = np.zeros((NCORES * av.shape[0], *av.shape[1:]), av.dtype)
        args.append(jax.device_put(z, sh))
    return args


def _run(in_maps):
    runner = _get_runner()
    args = _device_args(runner, in_maps)
    outs = runner["fn"](*args)
    return {name: np.asarray(outs[i])
            for i, name in enumerate(runner["out_names"])}


def bench(in_maps, iters=10):
    """Time repeated executions with device-resident inputs; returns
    (min_s, mean_s) per execution (includes axon RPC overhead)."""
    import time as _time
    import jax
    runner = _get_runner()
    args = _device_args(runner, in_maps)
    outs = runner["fn"](*args)           # warm-up/compile
    jax.block_until_ready(outs)
    times = []
    for _ in range(iters):
        t0 = _time.perf_counter()
        outs = runner["fn"](*args)
        jax.block_until_ready(outs)
        times.append(_time.perf_counter() - t0)
    return min(times), sum(times) / len(times)


# --------------------------------------------------------------------------
# entry point
# --------------------------------------------------------------------------

def kernel(**inputs):
    in_maps = _host_prep(**inputs)
    res = _run(in_maps)
    full = res["outT"].reshape(NCORES, 2, BC, 128, G).astype(np.float32)
    full = np.concatenate([full[c] for c in range(NCORES)], axis=1)
    full = np.ascontiguousarray(full.transpose(0, 1, 3, 2))  # [2, B, G, E]
    return (full[0], full[1])


# revision 7
# speedup vs baseline: 1.4404x; 1.0051x over previous
"""Trainium2 Bass kernel for nn_GAT_Solution (GNN message passing, 8-core data parallel).

Sharding: batch dim across 8 cores (4 batches each); small params replicated.
Host does integer index prep + weight layout/scale folding only; all float
compute runs on device.

v2 design (vs baseline):
  - K projection runs in fp8e4 (eg gathered embeddings and Wk both f8 with
    power-of-2 scales folded into host-prepared weights); same PE rate but
    halves the eg HBM traffic.
  - The mix-MLP hidden layer is ONE DoubleRow fp8 matmul per 512-col slice:
    tile0 = combo x prod8 (prod8 = q*k scaled by PS, written f8 by the DVE),
    tile1 = rank-1 edge-cost term (ec8 row rides in partition 0 of a
    once-zeroed carrier; stationary tile1 is w1bo in row 0). This removes the
    80 separate rank-1 ec matmuls entirely; relu evacuation applies the
    1/(PS*CS) dequant via the activation scale.
  - The softmax weighted sum runs in NATURAL node layout: w_n [128,8,S] f16
    comes straight out of the softmax (no transpose-back, no partition
    broadcasts, no PE ones-matmuls); pm_s = eg_nat8_s * w_n[:,:,s] broadcast
    along the inner free axis; f16 add tree; 8 f16 PE transposes + one DVE
    copy produce accT for the GRU. The embedding fp8 scale ES cancels via
    WihT pre-divided by ES on host.
  - GRU stays f16 (fp8 gates measurably break the 2e-2 budget).
  - Elementwise ops spread across DVE/Scalar/GpSimd by measured budget.
"""

import os
import numpy as np
import ml_dtypes

S, B, G, E, NH, KD, MSH = 10, 32, 1000, 128, 8, 16, 16
NCORES = 8
BC = B // NCORES          # 4 batches per core
GP = 1024                 # padded node count

# fp8 power-of-2 scales (folded into host-prepared weights)
ES = 32.0                 # embedding scale (egT8, eg_nat8)
SWK = 64.0                # Wk scale
PS = 16.0                 # prod8 scale (folded into WqT)
CS = 16.0                 # combo scale
SE = 64.0                 # edge-cost scale

_RUN_STATE = {}


# --------------------------------------------------------------------------
# device program
# --------------------------------------------------------------------------

def _build_program():
    import contextlib
    import concourse.bass as bass
    import concourse.bacc as bacc
    import concourse.tile as tile
    from concourse import mybir

    dt = mybir.dt
    AF = mybir.ActivationFunctionType
    OP = mybir.AluOpType
    AX = mybir.AxisListType
    DR = mybir.MatmulPerfMode.DoubleRow

    nc = bacc.Bacc("TRN2", target_bir_lowering=False, debug=False,
                   enable_asserts=False)

    def inp(name, shape, dtype):
        return nc.dram_tensor(name, list(shape), dtype, kind="ExternalInput").ap()

    embT   = inp("embT",   (BC, 128, G), dt.float16)
    egT8   = inp("egT8",   (BC, S, 128, GP), dt.float8e4)
    egn16  = inp("egn16",  (BC, S, 128, 8, 128), dt.float16)
    ecr8   = inp("ecr8",   (BC, S, GP), dt.float8e4)
    succn  = inp("succn",  (BC, 128, 8, S), dt.float32)
    cinv   = inp("cinv",   (BC, 128, 8, S), dt.float32)
    soldT  = inp("soldT",  (BC, 128, G), dt.float16)
    invc   = inp("invc",   (BC, 128, S), dt.float32)
    c0invc = inp("c0invc", (BC, 128, S), dt.float32)
    WqT    = inp("WqT",    (128, 128), dt.float16)
    WkT8   = inp("WkT8",   (128, 128), dt.float8e4)
    cw8    = inp("cw8",    (128, 2, 128), dt.float8e4)
    b1f    = inp("b1f",    (128, 1), dt.float32)
    coef   = inp("coef",   (128, 32), dt.float16)
    ident  = inp("ident",  (128, 128), dt.float32)
    ident16 = inp("ident16", (128, 128), dt.float16)
    WihT   = inp("WihT",   (128, 384), dt.float16)
    WhhT   = inp("WhhT",   (128, 384), dt.float16)
    gbsum  = inp("gbsum",  (128, 2), dt.float32)
    bihn   = inp("bihn",   (128, 1), dt.float32)
    bhhn   = inp("bhhn",   (128, 1), dt.float32)

    outT = nc.dram_tensor("outT", [2, BC, 128, G], dt.float16,
                          kind="ExternalOutput").ap()

    DQ = 1.0 / (PS * CS)   # mix dequant, applied in the relu activation

    with tile.TileContext(nc) as tc:
        with contextlib.ExitStack() as ctx:
            cpool = ctx.enter_context(tc.tile_pool(name="consts", bufs=1))
            io = ctx.enter_context(tc.tile_pool(name="io", bufs=2))
            gat = ctx.enter_context(tc.tile_pool(name="gat", bufs=3))
            egn = ctx.enter_context(tc.tile_pool(name="egn", bufs=22))
            work = ctx.enter_context(tc.tile_pool(name="work", bufs=2))
            ms1p = ctx.enter_context(tc.tile_pool(name="ms1p", bufs=3))
            sm = ctx.enter_context(tc.tile_pool(name="sm", bufs=2))
            rowa = ctx.enter_context(tc.tile_pool(name="rowa", bufs=2))
            pmp = ctx.enter_context(tc.tile_pool(name="pmp", bufs=11))
            tre = ctx.enter_context(tc.tile_pool(name="tre", bufs=1))
            gru = ctx.enter_context(tc.tile_pool(name="gru", bufs=1))
            psa = ctx.enter_context(
                tc.tile_pool(name="psa", bufs=1, space="PSUM"))
            psb = ctx.enter_context(
                tc.tile_pool(name="psb", bufs=2, space="PSUM"))
            psc = ctx.enter_context(
                tc.tile_pool(name="psc", bufs=1, space="PSUM"))

            def const(ap_, dtype, tag):
                t = cpool.tile(list(ap_.shape), dtype, tag=tag)
                nc.sync.dma_start(t[:], ap_)
                return t

            WqT_t = const(WqT, dt.float16, "cWqT")
            WkT8_t = const(WkT8, dt.float8e4, "cWkT8")
            cw8_t = const(cw8, dt.float8e4, "ccw8")
            b1f_t = const(b1f, dt.float32, "cb1f")
            coef_t = const(coef, dt.float16, "ccoef")
            ident_t = const(ident, dt.float32, "cident")
            ident16_t = const(ident16, dt.float16, "cident16")
            WihT_t = const(WihT, dt.float16, "cWih")
            WhhT_t = const(WhhT, dt.float16, "cWhh")
            gbsum_t = const(gbsum, dt.float32, "cgb")
            bihn_t = const(bihn, dt.float32, "cbihn")
            bhhn_t = const(bhhn, dt.float32, "cbhhn")

            # DoubleRow rhs ring: 3 persistent buffers (manually cycled so
            # the carrier zeros survive reuse).  Only [:,0,:] (prod8) and
            # [0:1,1,:] (ec row) are rewritten per use; the tile framework's
            # dependency tracking on the shared tensors enforces the ring.
            dr_bufs = [cpool.tile([128, 2, GP], dt.float8e4, tag=f"drb{i}",
                                  name=f"drb{i}")
                       for i in range(3)]
            for t in dr_bufs:
                nc.vector.memset(t[:, 1, :], 0.0)
            dr_ctr = [0]

            def mix_head(b):
                """Batch-b prologue: Q projection + the tail's small DMAs."""
                embT_t = io.tile([128, G], dt.float16, tag="embT")
                nc.sync.dma_start(embT_t[:], embT[b])

                qt_ps = psb.tile([128, GP], dt.float32, tag="mm")
                nc.tensor.matmul(qt_ps[:, 0:512], WqT_t[:], embT_t[:, 0:512],
                                 start=True, stop=True)
                nc.tensor.matmul(qt_ps[:, 512:G], WqT_t[:],
                                 embT_t[:, 512:G], start=True, stop=True)
                qt16 = work.tile([128, GP], dt.float16, tag="qt16")
                nc.vector.memset(qt16[:, G:GP], 0.0)
                nc.scalar.copy(qt16[:, 0:G], qt_ps[:, 0:G])

                st = {}
                st["invb"] = sm.tile([128, S], dt.float32, tag="invb",
                                     name=f"invb_{b}")
                nc.sync.dma_start(st["invb"][:], invc[b])
                st["c0b"] = sm.tile([128, S], dt.float32, tag="c0b",
                                    name=f"c0b_{b}")
                nc.sync.dma_start(st["c0b"][:], c0invc[b])
                st["sn"] = sm.tile([128, 8, S], dt.float32, tag="succn",
                                   name=f"sn_{b}")
                nc.sync.dma_start(st["sn"][:], succn[b])
                st["cnv"] = sm.tile([128, 8, S], dt.float32, tag="cinv",
                                    name=f"cnv_{b}")
                nc.sync.dma_start(st["cnv"][:], cinv[b])
                st["sold"] = gru.tile([128, GP], dt.float16,
                                      tag=f"sold{b % 2}", name=f"sold_{b}")
                nc.sync.dma_start(st["sold"][:, 0:G], soldT[b])
                nc.vector.memset(st["sold"][:, G:GP], 0.0)

                # eq depends only on the successor indices -- compute it here,
                # off the tail's critical chain
                eq = sm.tile([128, 8, S, S], dt.float32, tag="eq",
                             name=f"eq_{b}")
                nc.vector.tensor_tensor(
                    eq[:],
                    st["sn"][:].unsqueeze(3).broadcast_to([128, 8, S, S]),
                    st["sn"][:].unsqueeze(2).broadcast_to([128, 8, S, S]),
                    OP.is_equal)
                st["eq"] = eq

                cost_sb = rowa.tile([S, GP], dt.float32, tag="costsb")
                nc.vector.memset(cost_sb[:, G:GP], 0.0)
                st["cost_sb"] = cost_sb
                st["qt16"] = qt16
                st["egn"] = []
                return st

            def mix_group(b, st, grp):
                """Mix MLP for a group of <=3 solutions.  Per solution: f8 K
                matmuls, prod8 (DVE) into the DoubleRow rhs tile, one DR
                matmul per 512-slice (combo + rank-1 ec fused), relu evac
                with dequant scale, then the coef e-row matmuls."""
                qt16 = st["qt16"]
                cost_sb = st["cost_sb"]
                e_ps = psa.tile([96, GP], dt.float32, tag="e1")
                # stage 1: the first two K matmuls run back-to-back so the
                # DVE prod muls are fed as early as possible (kg ring = 2)
                kgs = {}
                drs = {}
                for gs, s in enumerate(grp):
                    eg_t = gat.tile([128, GP], dt.float8e4, tag="eg")
                    nc.sync.dma_start(eg_t[:], egT8[b, s])
                    egn_t = egn.tile([128, 8, 128], dt.float16, tag="egn")
                    nc.sync.dma_start(egn_t[:], egn16[b, s])
                    st["egn"].append(egn_t)
                    dr_t = dr_bufs[dr_ctr[0] % 3]
                    dr_ctr[0] += 1
                    drs[s] = dr_t
                    nc.sync.dma_start(dr_t[0:1, 1, :], ecr8[b, s].unsqueeze(0))
                    if gs < 2:
                        kg_ps = psb.tile([128, GP], dt.float32, tag="mm",
                                         name=f"kg_{b}_{s}")
                        nc.tensor.matmul(kg_ps[:, 0:512], WkT8_t[:],
                                         eg_t[:, 0:512], start=True, stop=True)
                        nc.tensor.matmul(kg_ps[:, 512:GP], WkT8_t[:],
                                         eg_t[:, 512:GP], start=True, stop=True)
                        nc.vector.tensor_mul(dr_t[:, 0, :], qt16[:], kg_ps[:])
                        kgs[s] = kg_ps
                    st.setdefault("eg_t", {})[s] = eg_t
                for gs, s in enumerate(grp):
                    dr_t = drs[s]
                    if gs >= 2:
                        eg_t = st["eg_t"][s]
                        kg_ps = psb.tile([128, GP], dt.float32, tag="mm",
                                         name=f"kg_{b}_{s}")
                        nc.tensor.matmul(kg_ps[:, 0:512], WkT8_t[:],
                                         eg_t[:, 0:512], start=True, stop=True)
                        nc.tensor.matmul(kg_ps[:, 512:GP], WkT8_t[:],
                                         eg_t[:, 512:GP], start=True, stop=True)
                        nc.vector.tensor_mul(dr_t[:, 0, :], qt16[:], kg_ps[:])
                    ms1_ps = psb.tile([128, GP], dt.float32, tag="mm",
                                      name=f"ms1_{b}_{s}")
                    for hh in range(2):
                        sl = slice(hh * 512, (hh + 1) * 512)
                        nc.tensor.matmul(ms1_ps[:, sl], cw8_t[:],
                                         dr_t[:, :, sl], start=True,
                                         stop=True, perf_mode=DR)
                    ms1 = ms1p.tile([128, GP], dt.float16, tag="ms1")
                    nc.scalar.activation(ms1[:], ms1_ps[:], AF.Relu,
                                         bias=b1f_t[:], scale=DQ)
                    base = gs * 32
                    for hh in range(2):
                        sl = slice(hh * 512, (hh + 1) * 512)
                        nc.tensor.matmul(e_ps[base:base + 32, sl],
                                         coef_t[:], ms1[:, sl],
                                         start=True, stop=True)
                np_ = (len(grp) - 1) * 32 + 1
                e_sb = work.tile([65, GP], dt.float32, tag="erow")
                nc.scalar.copy(e_sb[0:np_, 0:G], e_ps[0:np_, 0:G])
                for gs, s in enumerate(grp):
                    nc.sync.dma_start(cost_sb[s:s + 1, 0:G],
                                      e_sb[gs * 32:gs * 32 + 1, 0:G])

            def tail_cn(b, st):
                """Raw e rows -> natural layout [128, 8, S] (PE transposes,
                emitted first so they aren't queued behind the next mix)."""
                cost_sb = st["cost_sb"]
                cn_ps = psc.tile([128, 8 * S], dt.float32, tag="ps2",
                                 name=f"cn_{b}")
                for blk in range(8):
                    nc.tensor.transpose(
                        cn_ps[:, blk * S:(blk + 1) * S],
                        cost_sb[:, blk * 128:(blk + 1) * 128],
                        ident_t[0:S, 0:S])
                st["cn_ps"] = cn_ps

            def tail_softmax(b, st):
                """Merged softmax over the <=10 edges per node,
                cost = e/costs + C0/costs.  Emits w_n [128, 8, S] f16 in
                natural node layout (partition = t%128, blk = t//128)."""
                invb, c0b, cnv = st["invb"], st["c0b"], st["cnv"]
                cn_ps = st["cn_ps"]
                craw = sm.tile([128, 8, S], dt.float32, tag="craw")
                nc.vector.tensor_mul(
                    craw[:], cn_ps[:].rearrange("p (a b) -> p a b", a=8),
                    invb[:].unsqueeze(1).broadcast_to([128, 8, S]))
                cost_n = sm.tile([128, 8, S], dt.float32, tag="costn")
                nc.vector.tensor_add(
                    cost_n[:], craw[:],
                    c0b[:].unsqueeze(1).broadcast_to([128, 8, S]))

                # ---- softmax with duplicate-successor merge (no max shift:
                # |merged cost| < 1 by construction)
                eq = st["eq"]
                mm_ = sm.tile([128, 8, S, S], dt.float32, tag="mmul")
                nc.vector.tensor_mul(
                    mm_[:], eq[:],
                    cost_n[:].unsqueeze(2).broadcast_to([128, 8, S, S]))
                m_t = sm.tile([128, 8, S], dt.float32, tag="mt")
                nc.vector.tensor_reduce(m_t[:], mm_[:], AX.X, OP.add)
                p_t = sm.tile([128, 8, S], dt.float32, tag="pt")
                nc.scalar.activation(p_t[:], m_t[:], AF.Exp)
                pc = sm.tile([128, 8, S], dt.float32, tag="pc")
                nc.vector.tensor_mul(pc[:], p_t[:], cnv[:])
                z_t = sm.tile([128, 8], dt.float32, tag="zt")
                nc.vector.tensor_reduce(z_t[:], pc[:], AX.X, OP.add)
                zr = sm.tile([128, 8], dt.float32, tag="zr")
                nc.vector.reciprocal_approx_fast(zr[:], z_t[:])
                w_n = sm.tile([128, 8, S], dt.float16, tag="wn")
                nc.vector.tensor_mul(
                    w_n[:], pc[:],
                    zr[:].unsqueeze(2).broadcast_to([128, 8, S]))
                st["w_n"] = w_n

            def tail_wsum(b, st, drain=False):
                """Weighted sum of successor embeddings, natural layout:
                pm_s = eg_nat8_s * w_n[:,:,s] (broadcast along d), f16 add
                tree, then 8 f16 transposes + one DVE copy -> accT [128, GP]
                (values carry the ES embedding scale; WihT is pre-divided)."""
                w_n = st["w_n"]
                egn_tiles = st["egn"]

                # during the pipeline drain (last batch) there is no mix
                # work contending for SBUF, so GpSimd sharing is a net win;
                # otherwise GpSimd traffic slows concurrent DVE ops.
                gmuls = set(range(6, S)) if drain else set()
                pm = [None] * S
                for s in sorted(range(S), key=lambda x: x not in gmuls):
                    pm_t = pmp.tile([128, 8, 128], dt.float16, tag="pm",
                                    name=f"pm_{s}")
                    eng = nc.gpsimd if s in gmuls else nc.vector
                    eng.tensor_mul(
                        pm_t[:], egn_tiles[s][:],
                        w_n[:, :, s].unsqueeze(2).broadcast_to([128, 8, 128]))
                    pm[s] = pm_t
                ge = nc.gpsimd if drain else nc.vector
                t67 = tre.tile([128, 8, 128], dt.float16, tag="t67")
                ge.tensor_add(t67[:], pm[6][:], pm[7][:])
                t89 = tre.tile([128, 8, 128], dt.float16, tag="t89")
                ge.tensor_add(t89[:], pm[8][:], pm[9][:])
                t01 = tre.tile([128, 8, 128], dt.float16, tag="t01")
                nc.vector.tensor_add(t01[:], pm[0][:], pm[1][:])
                t23 = tre.tile([128, 8, 128], dt.float16, tag="t23")
                nc.vector.tensor_add(t23[:], pm[2][:], pm[3][:])
                t45 = tre.tile([128, 8, 128], dt.float16, tag="t45")
                nc.vector.tensor_add(t45[:], pm[4][:], pm[5][:])
                a0 = tre.tile([128, 8, 128], dt.float16, tag="a0")
                nc.vector.tensor_add(a0[:], t01[:], t23[:])
                a1 = tre.tile([128, 8, 128], dt.float16, tag="a1")
                ge.tensor_add(a1[:], t67[:], t89[:])
                a2 = tre.tile([128, 8, 128], dt.float16, tag="t23")
                nc.vector.tensor_add(a2[:], a0[:], t45[:])
                accn = tre.tile([128, 8, 128], dt.float16, tag="accn")
                nc.vector.tensor_add(accn[:], a2[:], a1[:])

                tr_ps = psc.tile([128, GP], dt.float16, tag="ps2",
                                 name=f"tr_{b}")
                for blk in range(8):
                    nc.tensor.transpose(
                        tr_ps[:, blk * 128:(blk + 1) * 128],
                        accn[:, blk, :], ident16_t[:])
                acc = gru.tile([128, GP], dt.float16, tag="accT")
                nc.vector.tensor_copy(acc[:], tr_ps[:])
                st["acc"] = acc

            def tail_gru(b, st):
                """GRU cell + elu epilogue (all f16 matmuls; acc carries the
                ES scale, cancelled by the host-scaled WihT)."""
                acc = st["acc"]
                sold_t = st["sold"]

                # r/z gates use different PSUM pools so they run in parallel
                gates = []
                for gidx, pool in ((0, psc), (1, psb)):   # r, z fused ih+hh
                    g_ps = pool.tile([128, GP], dt.float32,
                                     tag="ps2" if pool is psc else "mm",
                                     name=f"g{gidx}_{b}")
                    wsl = slice(gidx * 128, (gidx + 1) * 128)
                    for sl in (slice(0, 512), slice(512, GP)):
                        nc.tensor.matmul(g_ps[:, sl], WihT_t[:, wsl],
                                         acc[:, sl], start=True, stop=False)
                        nc.tensor.matmul(g_ps[:, sl], WhhT_t[:, wsl],
                                         sold_t[:, sl], start=False, stop=True)
                    gt = gru.tile([128, GP], dt.float16, tag=f"gate{gidx}")
                    nc.scalar.activation(gt[:], g_ps[:], AF.Sigmoid,
                                         bias=gbsum_t[:, gidx:gidx + 1])
                    gates.append(gt)
                r_t, z_g = gates

                ghn_ps = psc.tile([128, GP], dt.float32, tag="ps2")
                for sl in (slice(0, 512), slice(512, GP)):
                    nc.tensor.matmul(ghn_ps[:, sl], WhhT_t[:, 256:384],
                                     sold_t[:, sl], start=True, stop=True)
                ghs = gru.tile([128, GP], dt.float16, tag="ghs")
                nc.vector.tensor_scalar_add(ghs[:], ghn_ps[:], bhhn_t[:])
                gin_ps = psb.tile([128, GP], dt.float32, tag="mm")
                for sl in (slice(0, 512), slice(512, GP)):
                    nc.tensor.matmul(gin_ps[:, sl], WihT_t[:, 256:384],
                                     acc[:, sl], start=True, stop=True)
                rh = gru.tile([128, GP], dt.float16, tag="rh")
                nc.vector.tensor_mul(rh[:], r_t[:], ghs[:])
                tn = gru.tile([128, GP], dt.float32, tag="tn")
                nc.vector.tensor_add(tn[:], gin_ps[:], rh[:])
                n_t = gru.tile([128, GP], dt.float16, tag="nt")
                nc.scalar.activation(n_t[:], tn[:], AF.Tanh, bias=bihn_t[:])

                # new = n + z*(h - n)
                d_t = gru.tile([128, GP], dt.float16, tag="dt")
                nc.vector.tensor_sub(d_t[:], sold_t[:], n_t[:])
                zd = gru.tile([128, GP], dt.float16, tag="zd")
                nc.vector.tensor_mul(zd[:], z_g[:], d_t[:])
                new_t = gru.tile([128, GP], dt.float16, tag="newt")
                nc.vector.tensor_add(new_t[:], n_t[:], zd[:])
                nc.sync.dma_start(outT[1, b], new_t[:, 0:G])

                # elu(new) = relu(new) + exp(min(new,0)) - 1
                m0 = gru.tile([128, GP], dt.float16, tag="dt2")
                nc.vector.tensor_scalar_min(m0[:], new_t[:], 0.0)
                ex = gru.tile([128, GP], dt.float16, tag="rh2")
                nc.scalar.activation(ex[:], m0[:], AF.Exp)
                rl = gru.tile([128, GP], dt.float16, tag="ghs2")
                nc.vector.tensor_sub(rl[:], new_t[:], m0[:])
                ex1 = gru.tile([128, GP], dt.float16, tag="nt2")
                nc.vector.tensor_scalar_add(ex1[:], ex[:], -1.0)
                el = gru.tile([128, GP], dt.float16, tag="el")
                nc.vector.tensor_add(el[:], ex1[:], rl[:])
                nc.sync.dma_start(outT[0, b], el[:, 0:G])

            # software pipeline: interleave batch b+1's PE-heavy mix chunks
            # with batch b's DVE/Scalar-heavy tail chunks so every engine's
            # in-order queue alternates between the two batches.
            GRPS = [(0, 1, 2), (3, 4), (5, 6, 7), (8, 9)]
            states = {0: mix_head(0)}
            for grp in GRPS:
                mix_group(0, states[0], grp)
            for b in range(BC):
                nb = b + 1
                tail_cn(b, states[b])
                if nb < BC:
                    states[nb] = mix_head(nb)
                    mix_group(nb, states[nb], GRPS[0])
                tail_softmax(b, states[b])
                if nb < BC:
                    mix_group(nb, states[nb], GRPS[1])
                tail_wsum(b, states[b], drain=(b == BC - 1))
                if nb < BC:
                    mix_group(nb, states[nb], GRPS[2])
                tail_gru(b, states.pop(b))
                if nb < BC:
                    mix_group(nb, states[nb], GRPS[3])

    nc.compile()
    return nc


# --------------------------------------------------------------------------
# host prep (integer index work + layout/scale staging only)
# --------------------------------------------------------------------------

def _host_prep(node_embed, solutions, costs, dist, solution_embed_old,
               Wq, Wk, mix1_weight, mix1_bias, mix2_weight, mix2_bias,
               norm_head_w, gru_w_ih, gru_w_hh, gru_b_ih, gru_b_hh):
    f32 = np.float32
    f16 = np.float16
    f8 = ml_dtypes.float8_e4m3

    sol = np.asarray(solutions).astype(np.int64)
    nxt = np.roll(sol, -1, axis=-1)
    # succ[s,b,i]: successor of node i in tour (s,b)
    succ = np.zeros((S, B, G), dtype=np.int64)
    s_idx = np.arange(S)[:, None, None]
    b_idx = np.arange(B)[None, :, None]
    succ[s_idx, b_idx, sol] = nxt

    node_embed = np.asarray(node_embed, f32)
    dist = np.asarray(dist, f32)
    sold = np.asarray(solution_embed_old, f32)
    costs = np.asarray(costs, f32)

    Wq = np.asarray(Wq, f32); Wk = np.asarray(Wk, f32)
    m1w = np.asarray(mix1_weight, f32)   # [H, 2, M]
    m1b = np.asarray(mix1_bias, f32)     # [H, M]
    m2w = np.asarray(mix2_weight, f32)   # [H, M, 1]
    m2b = np.asarray(mix2_bias, f32)     # [H, 1]
    nhw = np.asarray(norm_head_w, f32)   # [H]
    wih = np.asarray(gru_w_ih, f32); whh = np.asarray(gru_w_hh, f32)
    bih = np.asarray(gru_b_ih, f32); bhh = np.asarray(gru_b_hh, f32)

    hm_h = np.repeat(np.arange(NH), MSH)          # head of each (h,m) slot
    dp_h = np.repeat(np.arange(NH), KD)           # head of each d' slot
    combo = np.where(dp_h[:, None] == hm_h[None, :],
                     (m1w[:, 0, :].reshape(-1) / 16.0)[None, :], 0.0)
    w1bo = m1w[:, 1, :].reshape(128)
    # DoubleRow stationary: tile0 = combo*CS, tile1 = w1bo carrier (row 0)
    cw8 = np.zeros((128, 2, 128), f8)
    cw8[:, 0, :] = (combo * CS).astype(f8)
    cw8[0, 1, :] = (w1bo * (CS * PS / SE)).astype(f8)

    coef = np.zeros((128, 32), f32)
    coef[:, 0] = (m2w[:, :, 0] * nhw[:, None]).reshape(128)
    c0 = float(np.dot(m2b[:, 0], nhw))
    gb = bih + bhh

    consts = dict(
        WqT=(np.ascontiguousarray(Wq.T) * (PS / (ES * SWK))).astype(f16),
        WkT8=(np.ascontiguousarray(Wk.T) * SWK).astype(f8),
        cw8=cw8,
        b1f=m1b.reshape(128, 1).astype(f32),
        coef=coef.astype(f16),
        ident=np.eye(128, dtype=f32),
        ident16=np.eye(128, dtype=f16),
        WihT=np.ascontiguousarray(wih.T).astype(f16),   # [128, 384]
        WhhT=np.ascontiguousarray(whh.T).astype(f16),
        gbsum=np.stack([gb[0:128], gb[128:256]], axis=1).astype(f32),
        bihn=bih[256:384].reshape(128, 1).astype(f32),
        bhhn=bhh[256:384].reshape(128, 1).astype(f32),
    )

    iv = np.arange(G)
    in_maps = []
    for c in range(NCORES):
        bs = slice(c * BC, (c + 1) * BC)
        ne = node_embed[bs]                        # [BC, G, E] f32
        ne8 = (ne * ES).astype(f8)                 # fp8 with ES scale
        sc = succ[:, bs, :]                        # [S, BC, G]

        egT8_ = np.zeros((BC, S, 128, GP), f8)
        egn16_ = np.zeros((BC, S, 128, 8, 128), f16)
        ecr8_ = np.zeros((BC, S, GP), f8)
        succn = np.zeros((BC, 128, 8, S), f32)
        cinv_ = np.ones((BC, 128, 8, S), f32)
        for bb in range(BC):
            scb = sc[:, bb, :]                     # [S, G]
            eqh = scb[None, :, :] == scb[:, None, :]
            cnt = eqh.sum(1)                       # [S, G] dup counts
            for s in range(S):
                sv = scb[s]                        # [G]
                g8 = ne8[bb][sv]                   # [G, 128] f8
                egT8_[bb, s, :, 0:G] = g8.T
                # natural layout: [t%128, t//128, d], zero-padded to 1024
                gpad = np.zeros((GP, 128), f16)
                gpad[0:G] = ne[bb][sv].astype(f16)
                egn16_[bb, s] = gpad.reshape(8, 128, 128).transpose(1, 0, 2)
                ecr8_[bb, s, 0:G] = (
                    dist[bs][bb][iv, sv] * SE).astype(f8)
                succn[bb, :, :, s] = 2000.0 + s
                succn[bb, iv % 128, iv // 128, s] = sv
                cinv_[bb, iv % 128, iv // 128, s] = 1.0 / cnt[s]

        im = dict(consts)
        im.update(
            embT=np.ascontiguousarray(
                ne.transpose(0, 2, 1)).astype(f16),
            egT8=egT8_,
            egn16=egn16_,
            ecr8=ecr8_,
            succn=succn,
            cinv=cinv_,
            soldT=np.ascontiguousarray(
                sold[bs].transpose(0, 2, 1)).astype(f16),
            invc=np.ascontiguousarray(np.broadcast_to(
                (1.0 / costs[:, bs]).T[:, None, :], (BC, 128, S))).astype(f32),
            c0invc=np.ascontiguousarray(np.broadcast_to(
                (c0 / costs[:, bs]).T[:, None, :], (BC, 128, S))).astype(f32),
        )
        in_maps.append(im)
    return in_maps


# --------------------------------------------------------------------------
# runner (mirrors concourse.bass2jax.run_bass_via_pjrt, but caches the jitted
# executable and keeps inputs device-resident so repeated runs can be timed)
# --------------------------------------------------------------------------

def _get_runner():
    if "runner" in _RUN_STATE:
        return _RUN_STATE["runner"]

    import jax
    from jax.sharding import Mesh, PartitionSpec
    from jax.experimental.shard_map import shard_map
    from concourse import mybir
    from concourse.bass2jax import (_bass_exec_p, install_neuronx_cc_hook,
                                    partition_id_tensor)

    if "nc" not in _RUN_STATE:
        _RUN_STATE["nc"] = _build_program()
    nc = _RUN_STATE["nc"]
    install_neuronx_cc_hook()

    pid_name = (nc.partition_id_tensor.name
                if nc.partition_id_tensor is not None else None)
    in_names, out_names, out_avals = [], [], []
    for alloc in nc.m.functions[0].allocations:
        if not isinstance(alloc, mybir.MemoryLocationSet):
            continue
        name = alloc.memorylocations[0].name
        if alloc.kind == "ExternalInput":
            if name != pid_name:
                in_names.append(name)
        elif alloc.kind == "ExternalOutput":
            out_names.append(name)
            out_avals.append(jax.core.ShapedArray(
                tuple(alloc.tensor_shape), mybir.dt.np(alloc.dtype)))
    n_params = len(in_names)
    all_names = in_names + out_names
    if pid_name is not None:
        all_names = all_names + [pid_name]

    def _body(*args):
        operands = list(args)
        if pid_name is not None:
            operands.append(partition_id_tensor())
        outs = _bass_exec_p.bind(
            *operands,
            out_avals=tuple(out_avals),
            in_names=tuple(all_names),
            out_names=tuple(out_names),
            lowering_input_output_aliases=(),
            sim_require_finite=True,
            sim_require_nnan=True,
            nc=nc,
        )
        return tuple(outs)

    devices = jax.devices()[:NCORES]
    mesh = Mesh(np.asarray(devices), ("core",))
    n_outs = len(out_names)
    sharded = jax.jit(
        shard_map(_body, mesh=mesh,
                  in_specs=(PartitionSpec("core"),) * (n_params + n_outs),
                  out_specs=(PartitionSpec("core"),) * n_outs,
                  check_rep=False),
        keep_unused=True,
    )

    runner = dict(fn=sharded, in_names=in_names, out_names=out_names,
                  out_avals=out_avals, mesh=mesh)
    _RUN_STATE["runner"] = runner
    return runner


def _device_args(runner, in_maps):
    import jax
    from jax.sharding import NamedSharding, PartitionSpec
    sh = NamedSharding(runner["mesh"], PartitionSpec("core"))
    args = []
    for i, name in enumerate(runner["in_names"]):
        arr = np.concatenate([np.asarray(m[name]) for m in in_maps], axis=0)
        args.append(jax.device_put(arr, sh))
    for av in runner["out_avals"]:
        z = np.zeros((NCORES * av.shape[0], *av.shape[1:]), av.dtype)
        args.append(jax.device_put(z, sh))
    return args


def _run(in_maps):
    runner = _get_runner()
    args = _device_args(runner, in_maps)
    outs = runner["fn"](*args)
    return {name: np.asarray(outs[i])
            for i, name in enumerate(runner["out_names"])}


def bench(in_maps, iters=10):
    """Time repeated executions with device-resident inputs; returns
    (min_s, mean_s) per execution (includes axon RPC overhead)."""
    import time as _time
    import jax
    runner = _get_runner()
    args = _device_args(runner, in_maps)
    outs = runner["fn"](*args)           # warm-up/compile
    jax.block_until_ready(outs)
    times = []
    for _ in range(iters):
        t0 = _time.perf_counter()
        outs = runner["fn"](*args)
        jax.block_until_ready(outs)
        times.append(_time.perf_counter() - t0)
    return min(times), sum(times) / len(times)


# --------------------------------------------------------------------------
# entry point
# --------------------------------------------------------------------------

def kernel(**inputs):
    in_maps = _host_prep(**inputs)
    res = _run(in_maps)
    full = res["outT"].reshape(NCORES, 2, BC, 128, G).astype(np.float32)
    full = np.concatenate([full[c] for c in range(NCORES)], axis=1)
    full = np.ascontiguousarray(full.transpose(0, 1, 3, 2))  # [2, B, G, E]
    return (full[0], full[1])


# revision 8
# speedup vs baseline: 1.4648x; 1.0170x over previous
"""Trainium2 Bass kernel for nn_GAT_Solution (GNN message passing, 8-core data parallel).

Sharding: batch dim across 8 cores (4 batches each); small params replicated.
Host does integer index prep + weight layout/scale folding only; all float
compute runs on device.

v2 design (vs baseline):
  - K projection runs in fp8e4 (eg gathered embeddings and Wk both f8 with
    power-of-2 scales folded into host-prepared weights); same PE rate but
    halves the eg HBM traffic.
  - The mix-MLP hidden layer is ONE DoubleRow fp8 matmul per 512-col slice:
    tile0 = combo x prod8 (prod8 = q*k scaled by PS, written f8 by the DVE),
    tile1 = rank-1 edge-cost term (ec8 row rides in partition 0 of a
    once-zeroed carrier; stationary tile1 is w1bo in row 0). This removes the
    80 separate rank-1 ec matmuls entirely; relu evacuation applies the
    1/(PS*CS) dequant via the activation scale.
  - The softmax weighted sum runs in NATURAL node layout: w_n [128,8,S] f16
    comes straight out of the softmax (no transpose-back, no partition
    broadcasts, no PE ones-matmuls); pm_s = eg_nat8_s * w_n[:,:,s] broadcast
    along the inner free axis; f16 add tree; 8 f16 PE transposes + one DVE
    copy produce accT for the GRU. The embedding fp8 scale ES cancels via
    WihT pre-divided by ES on host.
  - GRU stays f16 (fp8 gates measurably break the 2e-2 budget).
  - Elementwise ops spread across DVE/Scalar/GpSimd by measured budget.
"""

import os
import numpy as np
import ml_dtypes

S, B, G, E, NH, KD, MSH = 10, 32, 1000, 128, 8, 16, 16
NCORES = 8
BC = B // NCORES          # 4 batches per core
GP = 1024                 # padded node count

# fp8 power-of-2 scales (folded into host-prepared weights)
ES = 32.0                 # embedding scale (egT8, eg_nat8)
SWK = 64.0                # Wk scale
PS = 16.0                 # prod8 scale (folded into WqT)
CS = 16.0                 # combo scale
SE = 64.0                 # edge-cost scale

_RUN_STATE = {}


# --------------------------------------------------------------------------
# device program
# --------------------------------------------------------------------------

def _build_program():
    import contextlib
    import concourse.bass as bass
    import concourse.bacc as bacc
    import concourse.tile as tile
    from concourse import mybir

    dt = mybir.dt
    AF = mybir.ActivationFunctionType
    OP = mybir.AluOpType
    AX = mybir.AxisListType
    DR = mybir.MatmulPerfMode.DoubleRow

    nc = bacc.Bacc("TRN2", target_bir_lowering=False, debug=False,
                   enable_asserts=False)

    def inp(name, shape, dtype):
        return nc.dram_tensor(name, list(shape), dtype, kind="ExternalInput").ap()

    embT   = inp("embT",   (BC, 128, G), dt.float16)
    egT8   = inp("egT8",   (BC, S, 128, GP), dt.float8e4)
    egn8   = inp("egn8",   (BC, S, 128, 8, 128), dt.float8e4)
    ecr8   = inp("ecr8",   (BC, S, GP), dt.float8e4)
    succn  = inp("succn",  (BC, 128, 8, S), dt.float32)
    cinv   = inp("cinv",   (BC, 128, 8, S), dt.float32)
    soldT  = inp("soldT",  (BC, 128, G), dt.float16)
    invc   = inp("invc",   (BC, 128, S), dt.float32)
    c0invc = inp("c0invc", (BC, 128, S), dt.float32)
    WqT    = inp("WqT",    (128, 128), dt.float16)
    WkT8   = inp("WkT8",   (128, 128), dt.float8e4)
    cw8    = inp("cw8",    (128, 2, 128), dt.float8e4)
    b1f    = inp("b1f",    (128, 1), dt.float32)
    coef   = inp("coef",   (128, 32), dt.float16)
    ident  = inp("ident",  (128, 128), dt.float32)
    ident16 = inp("ident16", (128, 128), dt.float16)
    WihT   = inp("WihT",   (128, 384), dt.float16)
    WhhT   = inp("WhhT",   (128, 384), dt.float16)
    gbsum  = inp("gbsum",  (128, 2), dt.float32)
    bihn   = inp("bihn",   (128, 1), dt.float32)
    bhhn   = inp("bhhn",   (128, 1), dt.float32)

    outT = nc.dram_tensor("outT", [2, BC, 128, G], dt.float16,
                          kind="ExternalOutput").ap()

    DQ = 1.0 / (PS * CS)   # mix dequant, applied in the relu activation

    with tile.TileContext(nc) as tc:
        with contextlib.ExitStack() as ctx:
            cpool = ctx.enter_context(tc.tile_pool(name="consts", bufs=1))
            io = ctx.enter_context(tc.tile_pool(name="io", bufs=2))
            gat = ctx.enter_context(tc.tile_pool(name="gat", bufs=3))
            egn = ctx.enter_context(tc.tile_pool(name="egn", bufs=22))
            work = ctx.enter_context(tc.tile_pool(name="work", bufs=2))
            ms1p = ctx.enter_context(tc.tile_pool(name="ms1p", bufs=3))
            sm = ctx.enter_context(tc.tile_pool(name="sm", bufs=2))
            rowa = ctx.enter_context(tc.tile_pool(name="rowa", bufs=2))
            pmp = ctx.enter_context(tc.tile_pool(name="pmp", bufs=11))
            tre = ctx.enter_context(tc.tile_pool(name="tre", bufs=1))
            gru = ctx.enter_context(tc.tile_pool(name="gru", bufs=1))
            psa = ctx.enter_context(
                tc.tile_pool(name="psa", bufs=1, space="PSUM"))
            psb = ctx.enter_context(
                tc.tile_pool(name="psb", bufs=2, space="PSUM"))
            psc = ctx.enter_context(
                tc.tile_pool(name="psc", bufs=1, space="PSUM"))

            def const(ap_, dtype, tag):
                t = cpool.tile(list(ap_.shape), dtype, tag=tag)
                nc.sync.dma_start(t[:], ap_)
                return t

            WqT_t = const(WqT, dt.float16, "cWqT")
            WkT8_t = const(WkT8, dt.float8e4, "cWkT8")
            cw8_t = const(cw8, dt.float8e4, "ccw8")
            b1f_t = const(b1f, dt.float32, "cb1f")
            coef_t = const(coef, dt.float16, "ccoef")
            ident_t = const(ident, dt.float32, "cident")
            ident16_t = const(ident16, dt.float16, "cident16")
            WihT_t = const(WihT, dt.float16, "cWih")
            WhhT_t = const(WhhT, dt.float16, "cWhh")
            gbsum_t = const(gbsum, dt.float32, "cgb")
            bihn_t = const(bihn, dt.float32, "cbihn")
            bhhn_t = const(bhhn, dt.float32, "cbhhn")

            # DoubleRow rhs ring: 3 persistent buffers (manually cycled so
            # the carrier zeros survive reuse).  Only [:,0,:] (prod8) and
            # [0:1,1,:] (ec row) are rewritten per use; the tile framework's
            # dependency tracking on the shared tensors enforces the ring.
            dr_bufs = [cpool.tile([128, 2, GP], dt.float8e4, tag=f"drb{i}",
                                  name=f"drb{i}")
                       for i in range(3)]
            for t in dr_bufs:
                nc.vector.memset(t[:, 1, :], 0.0)
            dr_ctr = [0]

            def mix_head(b):
                """Batch-b prologue: Q projection + the tail's small DMAs."""
                embT_t = io.tile([128, G], dt.float16, tag="embT")
                nc.sync.dma_start(embT_t[:], embT[b])

                qt_ps = psb.tile([128, GP], dt.float32, tag="mm")
                nc.tensor.matmul(qt_ps[:, 0:512], WqT_t[:], embT_t[:, 0:512],
                                 start=True, stop=True)
                nc.tensor.matmul(qt_ps[:, 512:G], WqT_t[:],
                                 embT_t[:, 512:G], start=True, stop=True)
                qt16 = work.tile([128, GP], dt.float16, tag="qt16")
                nc.vector.memset(qt16[:, G:GP], 0.0)
                nc.scalar.copy(qt16[:, 0:G], qt_ps[:, 0:G])

                st = {}
                st["invb"] = sm.tile([128, S], dt.float32, tag="invb",
                                     name=f"invb_{b}")
                nc.sync.dma_start(st["invb"][:], invc[b])
                st["c0b"] = sm.tile([128, S], dt.float32, tag="c0b",
                                    name=f"c0b_{b}")
                nc.sync.dma_start(st["c0b"][:], c0invc[b])
                st["sn"] = sm.tile([128, 8, S], dt.float32, tag="succn",
                                   name=f"sn_{b}")
                nc.sync.dma_start(st["sn"][:], succn[b])
                st["cnv"] = sm.tile([128, 8, S], dt.float32, tag="cinv",
                                    name=f"cnv_{b}")
                nc.sync.dma_start(st["cnv"][:], cinv[b])
                st["sold"] = gru.tile([128, GP], dt.float16,
                                      tag=f"sold{b % 2}", name=f"sold_{b}")
                nc.sync.dma_start(st["sold"][:, 0:G], soldT[b])
                nc.vector.memset(st["sold"][:, G:GP], 0.0)

                cost_sb = rowa.tile([S, GP], dt.float32, tag="costsb")
                nc.vector.memset(cost_sb[:, G:GP], 0.0)
                st["cost_sb"] = cost_sb
                st["qt16"] = qt16
                st["egn"] = []
                return st

            def mix_group(b, st, grp):
                """Mix MLP for a group of <=3 solutions.  Per solution: f8 K
                matmuls, prod8 (DVE) into the DoubleRow rhs tile, one DR
                matmul per 512-slice (combo + rank-1 ec fused), relu evac
                with dequant scale, then the coef e-row matmuls."""
                qt16 = st["qt16"]
                cost_sb = st["cost_sb"]
                e_ps = psa.tile([96, GP], dt.float32, tag="e1")
                for gs, s in enumerate(grp):
                    eg_t = gat.tile([128, GP], dt.float8e4, tag="eg")
                    nc.sync.dma_start(eg_t[:], egT8[b, s])
                    egn_t = egn.tile([128, 8, 128], dt.float8e4, tag="egn")
                    nc.sync.dma_start(egn_t[:], egn8[b, s])
                    st["egn"].append(egn_t)

                    dr_t = dr_bufs[dr_ctr[0] % 3]
                    dr_ctr[0] += 1
                    nc.sync.dma_start(dr_t[0:1, 1, :], ecr8[b, s].unsqueeze(0))

                    kg_ps = psb.tile([128, GP], dt.float32, tag="mm")
                    nc.tensor.matmul(kg_ps[:, 0:512], WkT8_t[:],
                                     eg_t[:, 0:512], start=True, stop=True)
                    nc.tensor.matmul(kg_ps[:, 512:GP], WkT8_t[:],
                                     eg_t[:, 512:GP], start=True, stop=True)
                    nc.vector.tensor_mul(dr_t[:, 0, :], qt16[:], kg_ps[:])

                    ms1_ps = psb.tile([128, GP], dt.float32, tag="mm")
                    for hh in range(2):
                        sl = slice(hh * 512, (hh + 1) * 512)
                        nc.tensor.matmul(ms1_ps[:, sl], cw8_t[:],
                                         dr_t[:, :, sl], start=True,
                                         stop=True, perf_mode=DR)
                    ms1 = ms1p.tile([128, GP], dt.float16, tag="ms1")
                    nc.scalar.activation(ms1[:], ms1_ps[:], AF.Relu,
                                         bias=b1f_t[:], scale=DQ)
                    base = gs * 32
                    for hh in range(2):
                        sl = slice(hh * 512, (hh + 1) * 512)
                        nc.tensor.matmul(e_ps[base:base + 32, sl],
                                         coef_t[:], ms1[:, sl],
                                         start=True, stop=True)
                np_ = (len(grp) - 1) * 32 + 1
                e_sb = work.tile([65, GP], dt.float32, tag="erow")
                nc.scalar.copy(e_sb[0:np_, 0:G], e_ps[0:np_, 0:G])
                for gs, s in enumerate(grp):
                    nc.sync.dma_start(cost_sb[s:s + 1, 0:G],
                                      e_sb[gs * 32:gs * 32 + 1, 0:G])

            def tail_cn(b, st):
                """Raw e rows -> natural layout [128, 8, S] (PE transposes,
                emitted first so they aren't queued behind the next mix).
                eq is emitted here (not in mix_head) so the DVE queue isn't
                blocked on the succn DMA during the pipeline fill."""
                eq = sm.tile([128, 8, S, S], dt.float32, tag="eq",
                             name=f"eq_{b}")
                nc.vector.tensor_tensor(
                    eq[:],
                    st["sn"][:].unsqueeze(3).broadcast_to([128, 8, S, S]),
                    st["sn"][:].unsqueeze(2).broadcast_to([128, 8, S, S]),
                    OP.is_equal)
                st["eq"] = eq
                cost_sb = st["cost_sb"]
                cn_ps = psc.tile([128, 8 * S], dt.float32, tag="ps2",
                                 name=f"cn_{b}")
                for blk in range(8):
                    nc.tensor.transpose(
                        cn_ps[:, blk * S:(blk + 1) * S],
                        cost_sb[:, blk * 128:(blk + 1) * 128],
                        ident_t[0:S, 0:S])
                st["cn_ps"] = cn_ps

            def tail_softmax(b, st):
                """Merged softmax over the <=10 edges per node,
                cost = e/costs + C0/costs.  Emits w_n [128, 8, S] f16 in
                natural node layout (partition = t%128, blk = t//128)."""
                invb, c0b, cnv = st["invb"], st["c0b"], st["cnv"]
                cn_ps = st["cn_ps"]
                craw = sm.tile([128, 8, S], dt.float32, tag="craw")
                nc.vector.tensor_mul(
                    craw[:], cn_ps[:].rearrange("p (a b) -> p a b", a=8),
                    invb[:].unsqueeze(1).broadcast_to([128, 8, S]))
                cost_n = sm.tile([128, 8, S], dt.float32, tag="costn")
                nc.vector.tensor_add(
                    cost_n[:], craw[:],
                    c0b[:].unsqueeze(1).broadcast_to([128, 8, S]))

                # ---- softmax with duplicate-successor merge (no max shift:
                # |merged cost| < 1 by construction)
                eq = st["eq"]
                mm_ = sm.tile([128, 8, S, S], dt.float32, tag="mmul")
                nc.vector.tensor_mul(
                    mm_[:], eq[:],
                    cost_n[:].unsqueeze(2).broadcast_to([128, 8, S, S]))
                m_t = sm.tile([128, 8, S], dt.float32, tag="mt")
                nc.vector.tensor_reduce(m_t[:], mm_[:], AX.X, OP.add)
                p_t = sm.tile([128, 8, S], dt.float32, tag="pt")
                nc.scalar.activation(p_t[:], m_t[:], AF.Exp)
                pc = sm.tile([128, 8, S], dt.float32, tag="pc")
                nc.vector.tensor_mul(pc[:], p_t[:], cnv[:])
                z_t = sm.tile([128, 8], dt.float32, tag="zt")
                nc.vector.tensor_reduce(z_t[:], pc[:], AX.X, OP.add)
                zr = sm.tile([128, 8], dt.float32, tag="zr")
                nc.vector.reciprocal_approx_fast(zr[:], z_t[:])
                w_n = sm.tile([128, 8, S], dt.float16, tag="wn")
                nc.vector.tensor_mul(
                    w_n[:], pc[:],
                    zr[:].unsqueeze(2).broadcast_to([128, 8, S]))
                st["w_n"] = w_n

            def tail_wsum(b, st):
                """Weighted sum of successor embeddings, natural layout:
                pm_s = eg_nat8_s * w_n[:,:,s] (broadcast along d), f16 add
                tree, then 8 f16 transposes + one DVE copy -> accT [128, GP]
                (values carry the ES embedding scale; WihT is pre-divided)."""
                w_n = st["w_n"]
                egn_tiles = st["egn"]

                pm = []
                for s in range(S):
                    pm_t = pmp.tile([128, 8, 128], dt.float16, tag="pm")
                    nc.vector.tensor_mul(
                        pm_t[:], egn_tiles[s][:],
                        w_n[:, :, s].unsqueeze(2).broadcast_to([128, 8, 128]))
                    pm.append(pm_t)
                t01 = tre.tile([128, 8, 128], dt.float16, tag="t01")
                nc.vector.tensor_add(t01[:], pm[0][:], pm[1][:])
                t23 = tre.tile([128, 8, 128], dt.float16, tag="t23")
                nc.vector.tensor_add(t23[:], pm[2][:], pm[3][:])
                t45 = tre.tile([128, 8, 128], dt.float16, tag="t45")
                nc.vector.tensor_add(t45[:], pm[4][:], pm[5][:])
                t67 = tre.tile([128, 8, 128], dt.float16, tag="t67")
                nc.vector.tensor_add(t67[:], pm[6][:], pm[7][:])
                t89 = tre.tile([128, 8, 128], dt.float16, tag="t89")
                nc.vector.tensor_add(t89[:], pm[8][:], pm[9][:])
                a0 = tre.tile([128, 8, 128], dt.float16, tag="a0")
                nc.vector.tensor_add(a0[:], t01[:], t23[:])
                a1 = tre.tile([128, 8, 128], dt.float16, tag="a1")
                nc.vector.tensor_add(a1[:], t45[:], t67[:])
                a2 = tre.tile([128, 8, 128], dt.float16, tag="t23")
                nc.vector.tensor_add(a2[:], a0[:], a1[:])
                accn = tre.tile([128, 8, 128], dt.float16, tag="accn")
                nc.vector.tensor_add(accn[:], a2[:], t89[:])

                tr_ps = psc.tile([128, GP], dt.float16, tag="ps2",
                                 name=f"tr_{b}")
                for blk in range(8):
                    nc.tensor.transpose(
                        tr_ps[:, blk * 128:(blk + 1) * 128],
                        accn[:, blk, :], ident16_t[:])
                acc = gru.tile([128, GP], dt.float16, tag="accT")
                nc.vector.tensor_copy(acc[:], tr_ps[:])
                st["acc"] = acc

            def gru_gates(b, st):
                """GRU r/z gate matmuls + sigmoids, emitted ahead of the
                next batch's final mix group so the gate chain isn't queued
                behind it."""
                acc = st["acc"]
                sold_t = st["sold"]
                gates = []
                for gidx, pool in ((0, psc), (1, psb)):   # r, z fused ih+hh
                    g_ps = pool.tile([128, GP], dt.float32,
                                     tag="ps2" if pool is psc else "mm",
                                     name=f"g{gidx}_{b}")
                    wsl = slice(gidx * 128, (gidx + 1) * 128)
                    for sl in (slice(0, 512), slice(512, GP)):
                        nc.tensor.matmul(g_ps[:, sl], WihT_t[:, wsl],
                                         acc[:, sl], start=True, stop=False)
                        nc.tensor.matmul(g_ps[:, sl], WhhT_t[:, wsl],
                                         sold_t[:, sl], start=False, stop=True)
                    gt = gru.tile([128, GP], dt.float16, tag=f"gate{gidx}")
                    nc.scalar.activation(gt[:], g_ps[:], AF.Sigmoid,
                                         bias=gbsum_t[:, gidx:gidx + 1])
                    gates.append(gt)
                st["gates"] = gates

            def tail_gru(b, st):
                """GRU cell + elu epilogue (f16 matmuls)."""
                acc = st["acc"]
                sold_t = st["sold"]
                r_t, z_g = st["gates"]

                ghn_ps = psc.tile([128, GP], dt.float32, tag="ps2")
                for sl in (slice(0, 512), slice(512, GP)):
                    nc.tensor.matmul(ghn_ps[:, sl], WhhT_t[:, 256:384],
                                     sold_t[:, sl], start=True, stop=True)
                ghs = gru.tile([128, GP], dt.float16, tag="ghs")
                nc.vector.tensor_scalar_add(ghs[:], ghn_ps[:], bhhn_t[:])
                gin_ps = psb.tile([128, GP], dt.float32, tag="mm")
                for sl in (slice(0, 512), slice(512, GP)):
                    nc.tensor.matmul(gin_ps[:, sl], WihT_t[:, 256:384],
                                     acc[:, sl], start=True, stop=True)
                rh = gru.tile([128, GP], dt.float16, tag="rh")
                nc.vector.tensor_mul(rh[:], r_t[:], ghs[:])
                tn = gru.tile([128, GP], dt.float32, tag="tn")
                nc.vector.tensor_add(tn[:], gin_ps[:], rh[:])
                n_t = gru.tile([128, GP], dt.float16, tag="nt")
                nc.scalar.activation(n_t[:], tn[:], AF.Tanh, bias=bihn_t[:])

                # new = n + z*(h - n)
                d_t = gru.tile([128, GP], dt.float16, tag="dt")
                nc.vector.tensor_sub(d_t[:], sold_t[:], n_t[:])
                zd = gru.tile([128, GP], dt.float16, tag="zd")
                nc.vector.tensor_mul(zd[:], z_g[:], d_t[:])
                new_t = gru.tile([128, GP], dt.float16, tag="newt")
                nc.vector.tensor_add(new_t[:], n_t[:], zd[:])
                nc.sync.dma_start(outT[1, b], new_t[:, 0:G])

                # elu(new) = relu(new) + exp(min(new,0)) - 1
                m0 = gru.tile([128, GP], dt.float16, tag="dt2")
                nc.vector.tensor_scalar_min(m0[:], new_t[:], 0.0)
                ex = gru.tile([128, GP], dt.float16, tag="rh2")
                nc.scalar.activation(ex[:], m0[:], AF.Exp)
                rl = gru.tile([128, GP], dt.float16, tag="ghs2")
                nc.vector.tensor_sub(rl[:], new_t[:], m0[:])
                ex1 = gru.tile([128, GP], dt.float16, tag="nt2")
                nc.vector.tensor_scalar_add(ex1[:], ex[:], -1.0)
                el = gru.tile([128, GP], dt.float16, tag="el")
                nc.vector.tensor_add(el[:], ex1[:], rl[:])
                nc.sync.dma_start(outT[0, b], el[:, 0:G])

            # software pipeline: interleave batch b+1's PE-heavy mix chunks
            # with batch b's DVE/Scalar-heavy tail chunks so every engine's
            # in-order queue alternates between the two batches.
            GRPS = [(0, 1, 2), (3, 4, 5), (6, 7, 8), (9,)]
            states = {0: mix_head(0)}
            for grp in GRPS:
                mix_group(0, states[0], grp)
            for b in range(BC):
                nb = b + 1
                tail_cn(b, states[b])
                if nb < BC:
                    states[nb] = mix_head(nb)
                    mix_group(nb, states[nb], GRPS[0])
                tail_softmax(b, states[b])
                if nb < BC:
                    mix_group(nb, states[nb], GRPS[1])
                    mix_group(nb, states[nb], GRPS[2])
                tail_wsum(b, states[b])
                gru_gates(b, states[b])
                if nb < BC:
                    mix_group(nb, states[nb], GRPS[3])
                tail_gru(b, states.pop(b))

    nc.compile()
    return nc


# --------------------------------------------------------------------------
# host prep (integer index work + layout/scale staging only)
# --------------------------------------------------------------------------

def _host_prep(node_embed, solutions, costs, dist, solution_embed_old,
               Wq, Wk, mix1_weight, mix1_bias, mix2_weight, mix2_bias,
               norm_head_w, gru_w_ih, gru_w_hh, gru_b_ih, gru_b_hh):
    f32 = np.float32
    f16 = np.float16
    f8 = ml_dtypes.float8_e4m3

    sol = np.asarray(solutions).astype(np.int64)
    nxt = np.roll(sol, -1, axis=-1)
    # succ[s,b,i]: successor of node i in tour (s,b)
    succ = np.zeros((S, B, G), dtype=np.int64)
    s_idx = np.arange(S)[:, None, None]
    b_idx = np.arange(B)[None, :, None]
    succ[s_idx, b_idx, sol] = nxt

    node_embed = np.asarray(node_embed, f32)
    dist = np.asarray(dist, f32)
    sold = np.asarray(solution_embed_old, f32)
    costs = np.asarray(costs, f32)

    Wq = np.asarray(Wq, f32); Wk = np.asarray(Wk, f32)
    m1w = np.asarray(mix1_weight, f32)   # [H, 2, M]
    m1b = np.asarray(mix1_bias, f32)     # [H, M]
    m2w = np.asarray(mix2_weight, f32)   # [H, M, 1]
    m2b = np.asarray(mix2_bias, f32)     # [H, 1]
    nhw = np.asarray(norm_head_w, f32)   # [H]
    wih = np.asarray(gru_w_ih, f32); whh = np.asarray(gru_w_hh, f32)
    bih = np.asarray(gru_b_ih, f32); bhh = np.asarray(gru_b_hh, f32)

    hm_h = np.repeat(np.arange(NH), MSH)          # head of each (h,m) slot
    dp_h = np.repeat(np.arange(NH), KD)           # head of each d' slot
    combo = np.where(dp_h[:, None] == hm_h[None, :],
                     (m1w[:, 0, :].reshape(-1) / 16.0)[None, :], 0.0)
    w1bo = m1w[:, 1, :].reshape(128)
    # DoubleRow stationary: tile0 = combo*CS, tile1 = w1bo carrier (row 0)
    cw8 = np.zeros((128, 2, 128), f8)
    cw8[:, 0, :] = (combo * CS).astype(f8)
    cw8[0, 1, :] = (w1bo * (CS * PS / SE)).astype(f8)

    coef = np.zeros((128, 32), f32)
    coef[:, 0] = (m2w[:, :, 0] * nhw[:, None]).reshape(128)
    c0 = float(np.dot(m2b[:, 0], nhw))
    gb = bih + bhh

    consts = dict(
        WqT=(np.ascontiguousarray(Wq.T) * (PS / (ES * SWK))).astype(f16),
        WkT8=(np.ascontiguousarray(Wk.T) * SWK).astype(f8),
        cw8=cw8,
        b1f=m1b.reshape(128, 1).astype(f32),
        coef=coef.astype(f16),
        ident=np.eye(128, dtype=f32),
        ident16=np.eye(128, dtype=f16),
        WihT=(np.ascontiguousarray(wih.T) / ES).astype(f16),   # [128, 384]
        WhhT=np.ascontiguousarray(whh.T).astype(f16),
        gbsum=np.stack([gb[0:128], gb[128:256]], axis=1).astype(f32),
        bihn=bih[256:384].reshape(128, 1).astype(f32),
        bhhn=bhh[256:384].reshape(128, 1).astype(f32),
    )

    iv = np.arange(G)
    in_maps = []
    for c in range(NCORES):
        bs = slice(c * BC, (c + 1) * BC)
        ne = node_embed[bs]                        # [BC, G, E] f32
        ne8 = (ne * ES).astype(f8)                 # fp8 with ES scale
        sc = succ[:, bs, :]                        # [S, BC, G]

        egT8_ = np.zeros((BC, S, 128, GP), f8)
        egn8_ = np.zeros((BC, S, 128, 8, 128), f8)
        ecr8_ = np.zeros((BC, S, GP), f8)
        succn = np.zeros((BC, 128, 8, S), f32)
        cinv_ = np.ones((BC, 128, 8, S), f32)
        for bb in range(BC):
            scb = sc[:, bb, :]                     # [S, G]
            eqh = scb[None, :, :] == scb[:, None, :]
            cnt = eqh.sum(1)                       # [S, G] dup counts
            for s in range(S):
                sv = scb[s]                        # [G]
                g8 = ne8[bb][sv]                   # [G, 128] f8
                egT8_[bb, s, :, 0:G] = g8.T
                # natural layout: [t%128, t//128, d], zero-padded to 1024
                gpad = np.zeros((GP, 128), f8)
                gpad[0:G] = g8
                egn8_[bb, s] = gpad.reshape(8, 128, 128).transpose(1, 0, 2)
                ecr8_[bb, s, 0:G] = (
                    dist[bs][bb][iv, sv] * SE).astype(f8)
                succn[bb, :, :, s] = 2000.0 + s
                succn[bb, iv % 128, iv // 128, s] = sv
                cinv_[bb, iv % 128, iv // 128, s] = 1.0 / cnt[s]

        im = dict(consts)
        im.update(
            embT=np.ascontiguousarray(
                ne.transpose(0, 2, 1)).astype(f16),
            egT8=egT8_,
            egn8=egn8_,
            ecr8=ecr8_,
            succn=succn,
            cinv=cinv_,
            soldT=np.ascontiguousarray(
                sold[bs].transpose(0, 2, 1)).astype(f16),
            invc=np.ascontiguousarray(np.broadcast_to(
                (1.0 / costs[:, bs]).T[:, None, :], (BC, 128, S))).astype(f32),
            c0invc=np.ascontiguousarray(np.broadcast_to(
                (c0 / costs[:, bs]).T[:, None, :], (BC, 128, S))).astype(f32),
        )
        in_maps.append(im)
    return in_maps


# --------------------------------------------------------------------------
# runner (mirrors concourse.bass2jax.run_bass_via_pjrt, but caches the jitted
# executable and keeps inputs device-resident so repeated runs can be timed)
# --------------------------------------------------------------------------

def _get_runner():
    if "runner" in _RUN_STATE:
        return _RUN_STATE["runner"]

    import jax
    from jax.sharding import Mesh, PartitionSpec
    from jax.experimental.shard_map import shard_map
    from concourse import mybir
    from concourse.bass2jax import (_bass_exec_p, install_neuronx_cc_hook,
                                    partition_id_tensor)

    if "nc" not in _RUN_STATE:
        _RUN_STATE["nc"] = _build_program()
    nc = _RUN_STATE["nc"]
    install_neuronx_cc_hook()

    pid_name = (nc.partition_id_tensor.name
                if nc.partition_id_tensor is not None else None)
    in_names, out_names, out_avals = [], [], []
    for alloc in nc.m.functions[0].allocations:
        if not isinstance(alloc, mybir.MemoryLocationSet):
            continue
        name = alloc.memorylocations[0].name
        if alloc.kind == "ExternalInput":
            if name != pid_name:
                in_names.append(name)
        elif alloc.kind == "ExternalOutput":
            out_names.append(name)
            out_avals.append(jax.core.ShapedArray(
                tuple(alloc.tensor_shape), mybir.dt.np(alloc.dtype)))
    n_params = len(in_names)
    all_names = in_names + out_names
    if pid_name is not None:
        all_names = all_names + [pid_name]

    def _body(*args):
        operands = list(args)
        if pid_name is not None:
            operands.append(partition_id_tensor())
        outs = _bass_exec_p.bind(
            *operands,
            out_avals=tuple(out_avals),
            in_names=tuple(all_names),
            out_names=tuple(out_names),
            lowering_input_output_aliases=(),
            sim_require_finite=True,
            sim_require_nnan=True,
            nc=nc,
        )
        return tuple(outs)

    devices = jax.devices()[:NCORES]
    mesh = Mesh(np.asarray(devices), ("core",))
    n_outs = len(out_names)
    sharded = jax.jit(
        shard_map(_body, mesh=mesh,
                  in_specs=(PartitionSpec("core"),) * (n_params + n_outs),
                  out_specs=(PartitionSpec("core"),) * n_outs,
                  check_rep=False),
        keep_unused=True,
    )

    runner = dict(fn=sharded, in_names=in_names, out_names=out_names,
                  out_avals=out_avals, mesh=mesh)
    _RUN_STATE["runner"] = runner
    return runner


def _device_args(runner, in_maps):
    import jax
    from jax.sharding import NamedSharding, PartitionSpec
    sh = NamedSharding(runner["mesh"], PartitionSpec("core"))
    args = []
    for i, name in enumerate(runner["in_names"]):
        arr = np.concatenate([np.asarray(m[name]) for m in in_maps], axis=0)
        args.append(jax.device_put(arr, sh))
    for av in runner["out_avals"]:
        z = np.zeros((NCORES * av.shape[0], *av.shape[1:]), av.dtype)
        args.append(jax.device_put(z, sh))
    return args


def _run(in_maps):
    runner = _get_runner()
    args = _device_args(runner, in_maps)
    outs = runner["fn"](*args)
    return {name: np.asarray(outs[i])
            for i, name in enumerate(runner["out_names"])}


def bench(in_maps, iters=10):
    """Time repeated executions with device-resident inputs; returns
    (min_s, mean_s) per execution (includes axon RPC overhead)."""
    import time as _time
    import jax
    runner = _get_runner()
    args = _device_args(runner, in_maps)
    outs = runner["fn"](*args)           # warm-up/compile
    jax.block_until_ready(outs)
    times = []
    for _ in range(iters):
        t0 = _time.perf_counter()
        outs = runner["fn"](*args)
        jax.block_until_ready(outs)
        times.append(_time.perf_counter() - t0)
    return min(times), sum(times) / len(times)


# --------------------------------------------------------------------------
# entry point
# --------------------------------------------------------------------------

def kernel(**inputs):
    in_maps = _host_prep(**inputs)
    res = _run(in_maps)
    full = res["outT"].reshape(NCORES, 2, BC, 128, G).astype(np.float32)
    full = np.concatenate([full[c] for c in range(NCORES)], axis=1)
    full = np.ascontiguousarray(full.transpose(0, 1, 3, 2))  # [2, B, G, E]
    return (full[0], full[1])
